# revision 1
# baseline (speedup 1.0000x reference)
"""Trainium2 Bass kernel for nn_DeTree (oblivious decision-tree / MoE routing).

Full-input contract: kernel(**inputs) takes the unsharded inputs and returns
the full [2048, 512] output.  Internally shards 2-way over batch x 4-way over
trees across 8 NeuronCores (SPMD, no collectives), runs a Tile/Bass kernel,
and reassembles on host.

Math (per core, B=1024 batch rows, T=128 trees, nd=640 feature-columns):
  E = exp(feat_attention)                 (softmax numerator; denominator is
                                           folded into a per-row affine)
  FV^T = E^T x^T                          (mm1, PE; x transposed on-chip)
  u    = a*FV + c0   (= sparsemoid pre-clip logit 0.5 + 0.5*tl)
  Lp   = ln(clip(u, eps, 1)), Ln = ln(clip(1-u, eps, 1))
  S_lo/S_hi = path sums of logs           (mm2, PE; 0/1 path matrices)
  e_lo = exp(S_lo) [8/tree], e_hi = exp(S_hi) [4/tree]
  z    = R2 . e_lo                        (mm3, PE; response folded in)
  out  = sum_hi e_hi * z                  (DVE mul + mm4 ones-reduce)
The leaf product over depth 5 is exp(sum of logs); leaves are split into
(lo: depths 0-2 -> 8 ids) x (hi: depths 3,4 -> 4 ids) so only 12 exps/tree
are needed instead of 32.  clip eps=1e-20 makes exp underflow to exactly 0
where the reference gate is exactly 0.
"""

import os
import sys

import numpy as np

for _p in ("/opt/trn_rl_repo", "/root/.axon_site/_ro/trn_rl_repo"):
    if os.path.isdir(_p) and _p not in sys.path:
        sys.path.append(_p)

import concourse.bass as bass
import concourse.masks as masks
import concourse.mybir as mybir
import concourse.tile as tile
from concourse.bass_utils import run_bass_kernel_spmd

F32 = mybir.dt.float32
F32R = mybir.dt.float32r
AF = mybir.ActivationFunctionType
ALU = mybir.AluOpType

# problem shape (hardcoded per contest contract)
B, F, N, D = 2048, 512, 512, 5
NLEAF = 32
MB, MT = 2, 4                     # batch x tree sharding (MB*MT = 8 cores)
BC, TC = B // MB, N // MT         # 1024, 128 per core
ND = TC * D                       # 640 feature-columns per core
NLO, NHI = 8, 4                   # leaf-id split sizes (lo: depths 0-2, hi: 3-4)
NROW_LO = 3 * TC                  # 384 permuted lo rows (3 tiles)
EPS = 1e-20

_CACHE = {}
LAST = None  # BassKernelResults of the most recent run (for profiling)
FIXUP_WAITS = True  # set False for CoreSim (it can't interp the sem pseudos)
PHASES = 99  # timing-bisect knob: 1=xT+mm1, 2=+gates/ln, 3=+mm2/exp, 4=full
# float32r (single-pass matmul, 1 cy/row vs 4 for fp32): crashes the exec
# unit on TRN2 hardware (NRT_EXEC_UNIT_UNRECOVERABLE) -- keep OFF.
USE_F32R = False
MMDT = F32
GATES_ON_GPSIMD = False  # gpsimd tensor_scalar measured ~120us per [128,1024] op


def _structure(path_map):
    """Derive path matrices + leaf regroup from the runtime path_map."""
    path = np.asarray(path_map).reshape(NLEAF, D)
    lo_t = [tuple(int(path[l, j]) for j in (0, 1, 2)) for l in range(NLEAF)]
    hi_t = [tuple(int(path[l, j]) for j in (3, 4)) for l in range(NLEAF)]
    lo_ids = sorted(set(lo_t))
    hi_ids = sorted(set(hi_t))
    assert len(lo_ids) <= NLO and len(hi_ids) <= NHI, "path_map does not factor"
    lo_of = {t: i for i, t in enumerate(lo_ids)}
    hi_of = {t: i for i, t in enumerate(hi_ids)}
    P_lo = np.zeros((2 * D, NLO), np.float32)
    for t, i in lo_of.items():
        for e in t:
            P_lo[e, i] += 1.0
    P_hi = np.zeros((2 * D, NHI), np.float32)
    for t, i in hi_of.items():
        for e in t:
            P_hi[e, i] += 1.0
    leaf_hi = np.array([hi_of[t] for t in hi_t], np.int64)
    leaf_lo = np.array([lo_of[t] for t in lo_t], np.int64)
    return P_lo, P_hi, leaf_hi, leaf_lo


def _perm():
    """Permuted nd order: (t,d) d in 0..2 for all trees, then d in 3..4."""
    p = []
    for t in range(TC):
        for d in (0, 1, 2):
            p.append(5 * t + d)
    for t in range(TC):
        for d in (3, 4):
            p.append(5 * t + d)
    return np.array(p, np.int64)


def _mm2_passes(P_lo, P_hi):
    """Host-built lhsT tiles for the path-sum matmuls.

    Returns (pb, sb_passes, sa_passes): pb [NPB,128,128];
    sb_passes[sigma] / sa_passes[alpha] are lists of (pb_idx, tau, sign).
    sign 1 -> rhs Lp, 0 -> rhs Ln.  S row conventions:
      S_lo out-tile sigma: partition 8*tl+lo, trees 16*sigma+tl
      S_hi out-tile alpha: partition 4*tl+hi, trees 32*alpha+tl
    """
    mats, sb_passes, sa_passes = [], [], []
    for sig in range(8):
        passes = []
        for s in (1, 0):
            by_tau = {}
            for tl in range(16):
                t = 16 * sig + tl
                for d in (0, 1, 2):
                    r = 3 * t + d
                    tau, k = r // 128, r % 128
                    m = by_tau.setdefault(tau, np.zeros((128, 128), np.float32))
                    for lo in range(NLO):
                        m[k, 8 * tl + lo] = P_lo[2 * d + s, lo]
            for tau in sorted(by_tau):
                passes.append((len(mats), tau, s))
                mats.append(by_tau[tau])
        sb_passes.append(passes)
    for al in range(4):
        passes = []
        for s in (1, 0):
            by_tau = {}
            for tl in range(32):
                t = 32 * al + tl
                for d in (3, 4):
                    r = NROW_LO + 2 * t + (d - 3)
                    tau, k = r // 128, r % 128
                    m = by_tau.setdefault(tau, np.zeros((128, 128), np.float32))
                    for hi in range(NHI):
                        m[k, 4 * tl + hi] = P_hi[2 * d + s, hi]
            for tau in sorted(by_tau):
                passes.append((len(mats), tau, s))
                mats.append(by_tau[tau])
        sa_passes.append(passes)
    return np.stack(mats), sb_passes, sa_passes


def _build_nc(npb, sb_passes, sa_passes):
    MMDT = F32R if USE_F32R else F32
    nc = bass.Bass()
    x_in = nc.dram_tensor("x", [BC, F], F32, kind="ExternalInput")
    a_in = nc.dram_tensor("fa", [F, ND], F32, kind="ExternalInput")
    # all small constants pre-laid host-side as [128, X] so each is ONE DMA
    pb_in = nc.dram_tensor("pb", [128, npb * 128], MMDT, kind="ExternalInput")
    r2_in = nc.dram_tensor("r2l", [128, 8 * 128], F32, kind="ExternalInput")
    on_in = nc.dram_tensor("onesd", [128, 4 * 128], MMDT, kind="ExternalInput")
    c0_in = nc.dram_tensor("c0", [128, 5], F32, kind="ExternalInput")
    c1_in = nc.dram_tensor("c1", [128, 5], F32, kind="ExternalInput")
    out_d = nc.dram_tensor("out_t", [TC, BC], F32, kind="ExternalOutput")

    deps = []  # (dependent BassInstruction, dependency BassInstruction)

    with tile.TileContext(nc) as tc:
        with (
            tc.tile_pool(name="const", bufs=1) as cpool,
            tc.tile_pool(name="big", bufs=1) as bpool,
            tc.tile_pool(name="work", bufs=2) as wpool,
            tc.tile_pool(name="out", bufs=1) as opool,
            tc.tile_pool(name="psum", bufs=3, space="PSUM") as pp,
            tc.tile_pool(name="psink", bufs=1, space="PSUM") as psink,
        ):
            # PE matmuls (incl. transpose-mode) only tolerate ONE sync wait
            # after walrus lowering; "touch" matmuls absorb producer-engine
            # waits into PE's vector clock ahead of the real matmuls.
            sink = psink.tile([1, 1], F32, tag="sink")

            def pe_touch(ap):
                if ap.dtype == F32R:
                    ap = ap.bitcast(F32)
                return nc.tensor.matmul(sink[:], ap, ap, start=True, stop=True,
                                        skip_group_check=True)

            # ---- constants (one DMA each) ----
            idt = cpool.tile([128, 128], F32, tag="idt")
            masks.make_identity(nc, idt[:])
            t_idt = pe_touch(idt[:, 0:1])
            onest = cpool.tile([128, 4 * 128], MMDT, tag="ones")
            nc.sync.dma_start(onest[:], on_in[:])
            t_on = pe_touch(onest[:, 0:1])
            pbt = cpool.tile([128, npb * 128], MMDT, tag="pb")
            nc.sync.dma_start(pbt[:], pb_in[:])
            t_pb = pe_touch(pbt[:, 0:1])
            r2t = cpool.tile([128, 8 * 128], MMDT, tag="r2")
            r2raw = cpool.tile([128, 8 * 128], F32, tag="r2raw")
            nc.sync.dma_start(r2raw[:], r2_in[:])
            nc.vector.tensor_copy(r2t[:], r2raw[:])
            t_r2 = pe_touch(r2t[:, 0:1])
            c0t = cpool.tile([128, 5], F32, tag="c0")
            nc.sync.dma_start(c0t[:], c0_in[:])
            c1t = cpool.tile([128, 5], F32, tag="c1")
            nc.sync.dma_start(c1t[:], c1_in[:])
            ones_col = cpool.tile([128, 1], MMDT, tag="onescol")
            ones_raw = cpool.tile([128, 1], F32, tag="onescolr")
            nc.vector.memset(ones_raw[:], 1.0)
            nc.vector.tensor_copy(ones_col[:], ones_raw[:])
            t_oc = pe_touch(ones_col[:])

            # ---- E = exp(A), column sums, a = c1/colsum ----
            et = bpool.tile([128, 4 * ND], MMDT, tag="E")
            for ft in range(4):
                araw = wpool.tile([128, ND], F32, tag="u")
                nc.sync.dma_start(araw[:], a_in[bass.ts(ft, 128), :])
                nc.scalar.activation(et[:, ND * ft:ND * (ft + 1)], araw[:], AF.Exp)
            cs_a = pp.tile([1, 512], F32, tag="big")
            cs_b = pp.tile([1, 128], F32, tag="big")
            for ft in range(4):
                st, sp = ft == 0, ft == 3
                m1 = nc.tensor.matmul(cs_a[:], ones_col[:].bitcast(F32),
                                      et[:, ND * ft:ND * ft + 512].bitcast(F32),
                                      start=st, stop=sp)
                m2 = nc.tensor.matmul(cs_b[:], ones_col[:].bitcast(F32),
                                      et[:, ND * ft + 512:ND * (ft + 1)].bitcast(F32),
                                      start=st, stop=sp)
                deps += [(m1, t_oc), (m2, t_oc)]
            invs = cpool.tile([1, ND], F32, tag="invs")
            nc.vector.reciprocal(invs[:, 0:512], cs_a[:])
            nc.vector.reciprocal(invs[:, 512:640], cs_b[:])
            a_sb = cpool.tile([128, 5], F32, tag="a_sb")
            for t in range(5):
                tp = pp.tile([128, 128], F32, tag="big")
                tr = nc.tensor.transpose(tp[:, 0:1], invs[0:1, bass.ts(t, 128)],
                                         idt[0:1, 0:1])
                deps.append((tr, t_idt))
                nc.vector.tensor_scalar(a_sb[:, t:t + 1], tp[:, 0:1],
                                        c1t[:, t:t + 1], None, ALU.mult)

            # ---- x^T via PE transpose ----
            xts = bpool.tile([128, 4 * BC], MMDT, tag="xT")
            xraw = bpool.tile([128, 8 * F], F32, tag="xraw")
            for bt in range(8 if PHASES >= 1 else 0):
                nc.sync.dma_start(xraw[:, F * bt:F * (bt + 1)],
                                  x_in[bass.ts(bt, 128), :])
            for ft in range(4 if PHASES >= 1 else 0):
                tp = pp.tile([128, BC], F32, tag="big")
                for bt in range(8):
                    tr = nc.tensor.transpose(
                        tp[:, bass.ts(bt, 128)],
                        xraw[:, F * bt + 128 * ft:F * bt + 128 * (ft + 1)],
                        idt[:])
                    deps.append((tr, t_idt))
                nc.vector.tensor_copy(xts[:, BC * ft:BC * (ft + 1)], tp[:])

            # ---- mm1 + gates + logs, per nd-tile ----
            lpt = bpool.tile([128, 5 * BC], MMDT, tag="Lp")
            lnt = bpool.tile([128, 5 * BC], MMDT, tag="Ln")
            t_u = []
            last_gate = None
            for t in range(5 if PHASES >= 1 else 0):
                fv = pp.tile([128, BC], F32, tag="big")
                for ft in range(4):
                    st, sp = ft == 0, ft == 3
                    for h in range(2):
                        nc.tensor.matmul(
                            fv[:, bass.ts(h, 512)],
                            et[:, ND * ft + 128 * t:ND * ft + 128 * (t + 1)],
                            xts[:, BC * ft + 512 * h:BC * ft + 512 * (h + 1)],
                            start=st, stop=sp)
                ut = wpool.tile([128, BC], F32, tag="u")
                nc.vector.tensor_scalar(ut[:], fv[:], a_sb[:, t:t + 1],
                                        c0t[:, t:t + 1], ALU.mult, ALU.add)
                t_u.append(pe_touch(ut[:, 0:1]))
                geng = nc.gpsimd if GATES_ON_GPSIMD else nc.vector
                upt = wpool.tile([128, BC], F32, tag="up")
                geng.tensor_scalar(upt[:], ut[:], 1.0, EPS, ALU.min, ALU.max)
                unt = wpool.tile([128, BC], F32, tag="un")
                geng.tensor_scalar(unt[:], ut[:], -1.0, 1.0,
                                   ALU.mult, ALU.add)
                geng.tensor_scalar(unt[:], unt[:], 1.0, EPS,
                                   ALU.min, ALU.max)
                if PHASES >= 2:
                    nc.scalar.activation(lpt[:, BC * t:BC * (t + 1)],
                                         upt[:], AF.Ln)
                    nc.scalar.activation(lnt[:, BC * t:BC * (t + 1)],
                                         unt[:], AF.Ln)
                last_gate = unt

            if PHASES < 3:
                out_sb = opool.tile([128, BC], F32, tag="osb")
                src_t = last_gate if PHASES < 2 else lpt
                if src_t is None:
                    nc.vector.memset(out_sb[:], 0.0)
                elif src_t is lpt:
                    nc.vector.tensor_copy(out_sb[:], src_t[:, 0:BC])
                else:
                    nc.vector.tensor_copy(out_sb[:], src_t[:])
                nc.sync.dma_start(out_d[:], out_sb[:])
                from concourse.tile import add_dep_helper as _adh
                for a, b in deps:
                    _adh(a.ins, b.ins, sync=False, reason="pre-sync")
                deps.clear()
            if PHASES < 3:
                pass  # skip back half entirely

            def lsrc(s, tau):
                src = lpt if s == 1 else lnt
                return src[:, BC * tau:BC * (tau + 1)]

            # ---- mm2 (path sums) + exp ----
            ebt = bpool.tile([128, 8 * BC], MMDT, tag="eB")
            for sg in range(8 if PHASES >= 3 else 0):
                sb = pp.tile([128, BC], F32, tag="big")
                passes = sb_passes[sg]
                for i, (pi, tau, s) in enumerate(passes):
                    st, sp = i == 0, i == len(passes) - 1
                    for h in range(2):
                        mm = nc.tensor.matmul(
                            sb[:, bass.ts(h, 512)],
                            pbt[:, bass.ts(pi, 128)],
                            lsrc(s, tau)[:, bass.ts(h, 512)],
                            start=st, stop=sp)
                        deps += [(mm, t_pb)] + [(mm, tu) for tu in t_u]
                nc.scalar.activation(ebt[:, BC * sg:BC * (sg + 1)], sb[:], AF.Exp)
            eat = bpool.tile([128, 4 * BC], F32, tag="eA")
            for al in range(4 if PHASES >= 3 else 0):
                sa = pp.tile([128, BC], F32, tag="big")
                passes = sa_passes[al]
                for i, (pi, tau, s) in enumerate(passes):
                    st, sp = i == 0, i == len(passes) - 1
                    for h in range(2):
                        mm = nc.tensor.matmul(
                            sa[:, bass.ts(h, 512)],
                            pbt[:, bass.ts(pi, 128)],
                            lsrc(s, tau)[:, bass.ts(h, 512)],
                            start=st, stop=sp)
                        deps += [(mm, t_pb)] + [(mm, tu) for tu in t_u]
                nc.scalar.activation(eat[:, BC * al:BC * (al + 1)], sa[:], AF.Exp)

            # ---- mm3 (z = R2 . e_lo), P = e_hi * z, mm4 (ones-reduce) ----
            if PHASES == 3:
                out_sb = opool.tile([128, BC], F32, tag="osb")
                nc.vector.tensor_copy(out_sb[:], ebt[:, 0:BC])
                nc.sync.dma_start(out_d[:], out_sb[:])
            outp = pp.tile([128, BC], F32, tag="big")
            t_prev = None
            for al in range(4 if PHASES >= 4 else 0):
                z = pp.tile([128, BC], F32, tag="big")
                for j in range(2):
                    sg = 2 * al + j
                    for h in range(2):
                        mm = nc.tensor.matmul(
                            z[:, bass.ts(h, 512)],
                            r2t[:, bass.ts(sg, 128)],
                            ebt[:, BC * sg + 512 * h:BC * sg + 512 * (h + 1)],
                            start=j == 0, stop=j == 1)
                        deps.append((mm, t_r2))
                        if t_prev is not None:
                            deps.append((mm, t_prev))
                pt = wpool.tile([128, BC], MMDT, tag="P")
                nc.vector.tensor_mul(pt[:], eat[:, BC * al:BC * (al + 1)], z[:])
                t_pt = pe_touch(pt[:, 0:1])
                for h in range(2):
                    mm = nc.tensor.matmul(
                        outp[:, bass.ts(h, 512)],
                        onest[:, bass.ts(al, 128)],
                        pt[:, bass.ts(h, 512)],
                        start=al == 0, stop=al == 3,
                        skip_group_check=True)
                    deps += [(mm, t_on), (mm, t_pt)]
                t_prev = t_pt
            if PHASES >= 4:
                out_sb = opool.tile([128, BC], F32, tag="osb")
                nc.vector.tensor_copy(out_sb[:], outp[:])
                nc.sync.dma_start(out_d[:], out_sb[:])

            from concourse.tile import add_dep_helper
            for a, b in deps:
                add_dep_helper(a.ins, b.ins, sync=False,
                               reason="PE pre-sync absorbs extra waits")
    if FIXUP_WAITS:
        _split_excess_waits(nc)
    return nc


def _split_excess_waits(nc):
    """Walrus codegen only fits ONE sync wait on PE Matmult and DMACopy
    instructions ("Too many sync wait commands").  Hoist the extras onto
    preceding same-engine InstEventSemaphore pseudos (one wait each), which
    the sequencer executes before the limited instruction."""
    exempt = {"InstEventSemaphore", "InstUnconditionalBranch",
              "InstISA", "InstHalt"}
    nfix = 0
    for fn in nc.m.functions:
        for bb in fn.blocks:
            il = bb.instructions
            out = []
            for inst in il:
                si = inst.sync_info
                lim = None if type(inst).__name__ in exempt else 1
                if si is not None and lim is not None and len(si.on_wait) > lim:
                    keep = list(si.on_wait[-lim:])
                    for w in si.on_wait[:-lim]:
                        nfix += 1
                        ev = mybir.InstEventSemaphore(
                            name=f"I-waitfix-{nfix}",
                            engine=inst.engine,
                            ins=[], outs=[],
                            sync_info=mybir.SyncInfo(on_wait=[w], on_update=[]),
                        )
                        ev.bass_nofuse = True
                        out.append(ev)
                    inst.sync_info = mybir.SyncInfo(
                        on_wait=keep, on_update=list(si.on_update))
                out.append(inst)
            il[:] = out
            assert len(bb.instructions) == len(out)
    return nfix


def _prep(path_map):
    key = (np.asarray(path_map).tobytes(), PHASES, GATES_ON_GPSIMD, USE_F32R)
    if key not in _CACHE:
        P_lo, P_hi, leaf_hi, leaf_lo = _structure(path_map)
        pb, sb_passes, sa_passes = _mm2_passes(P_lo, P_hi)
        nc = _build_nc(pb.shape[0], sb_passes, sa_passes)
        _CACHE[key] = (pb, leaf_hi, leaf_lo, nc)
    return _CACHE[key]


def build_in_maps(x, feat_attention, feature_thresholds, log_temperatures,
                  response, path_map):
    x = np.ascontiguousarray(np.asarray(x, np.float32))
    fa = np.asarray(feat_attention, np.float32)
    thr = np.asarray(feature_thresholds, np.float32)
    lt = np.asarray(log_temperatures, np.float32)
    resp = np.asarray(response, np.float32).reshape(N, NLEAF)
    pb, leaf_hi, leaf_lo, nc = _prep(path_map)

    perm = _perm()
    invtemp = np.exp(-lt)                        # [N, D] host weight prep
    c1_all = (0.5 * invtemp).reshape(N * D)
    c0_all = (0.5 - 0.5 * thr * invtemp).reshape(N * D)
    # R2[n, hi, lo] = sum of response over leaves in that (hi, lo) group
    R2 = np.zeros((N, NHI, NLO), np.float32)
    np.add.at(R2, (slice(None), leaf_hi, leaf_lo), resp)

    onesd = np.zeros((4, 128, 128), np.float32)
    for al in range(4):
        for tl in range(32):
            for hi in range(NHI):
                onesd[al, 4 * tl + hi, 32 * al + tl] = 1.0
    onesd = np.ascontiguousarray(onesd.transpose(1, 0, 2).reshape(128, 512))
    pb2 = np.ascontiguousarray(
        pb.transpose(1, 0, 2).reshape(128, pb.shape[0] * 128))

    in_maps = []
    for c in range(8):
        bi, ti = c // MT, c % MT
        t0 = ti * TC
        cols = t0 * D + perm                      # permuted global nd columns
        c0 = np.ascontiguousarray(c0_all[cols].reshape(5, 128).T)
        c1 = np.ascontiguousarray(c1_all[cols].reshape(5, 128).T)
        r2l = np.zeros((8, 128, 128), np.float32)
        for sg in range(8):
            off = 64 * (sg % 2)
            for tl in range(16):
                t = t0 + 16 * sg + tl
                for hi in range(NHI):
                    for lo in range(NLO):
                        r2l[sg, 8 * tl + lo, off + 4 * tl + hi] = R2[t, hi, lo]
        r2l = np.ascontiguousarray(r2l.transpose(1, 0, 2).reshape(128, 1024))
        in_maps.append({
            "x": np.ascontiguousarray(x[bi * BC:(bi + 1) * BC]),
            "fa": np.ascontiguousarray(fa[:, t0 * D:(t0 + TC) * D][:, perm]),
            "pb": pb2,
            "r2l": r2l,
            "onesd": onesd,
            "c0": c0,
            "c1": c1,
        })
    return in_maps, nc


def kernel(x, feat_attention, feature_thresholds, log_temperatures,
           response, path_map):
    in_maps, nc = build_in_maps(x, feat_attention, feature_thresholds,
                                log_temperatures, response, path_map)
    res = run_bass_kernel_spmd(nc, in_maps, list(range(8)))
    global LAST
    LAST = res
    out = np.empty((B, N), np.float32)
    for c in range(8):
        bi, ti = c // MT, c % MT
        out[bi * BC:(bi + 1) * BC, ti * TC:(ti + 1) * TC] = res.results[c]["out_t"].T
    return out



# revision 24
# speedup vs baseline: 301.9549x; 301.9549x over previous
"""Trainium2 Bass kernel for nn_DeTree (oblivious decision-tree / MoE routing).

Full-input contract: kernel(**inputs) takes the unsharded inputs and returns
the full [2048, 512] output.  Internally shards 2-way over batch x 4-way over
trees across 8 NeuronCores (SPMD, no collectives), runs a Tile/Bass kernel,
and reassembles on host.

Math (per core, B=1024 batch rows, T=128 trees, nd=640 feature-columns):
  E = exp(feat_attention)                 (softmax numerator; denominator is
                                           folded into a per-row affine)
  FV^T = E^T x^T                          (mm1, PE; x transposed on-chip)
  u    = a*FV + c0   (= sparsemoid pre-clip logit 0.5 + 0.5*tl)
  Lp   = ln(clip(u, eps, 1)), Ln = ln(clip(1-u, eps, 1))
  S_lo/S_hi = path sums of logs           (mm2, PE; 0/1 path matrices)
  e_lo = exp(S_lo) [8/tree], e_hi = exp(S_hi) [4/tree]
  z    = R2 . e_lo                        (mm3, PE; response folded in)
  out  = sum_hi e_hi * z                  (DVE mul + mm4 ones-reduce)
The leaf product over depth 5 is exp(sum of logs); leaves are split into
(lo: depths 0-2 -> 8 ids) x (hi: depths 3,4 -> 4 ids) so only 12 exps/tree
are needed instead of 32.  clip eps=1e-20 makes exp underflow to exactly 0
where the reference gate is exactly 0.
"""

import os
import sys

import numpy as np

for _p in ("/opt/trn_rl_repo", "/root/.axon_site/_ro/trn_rl_repo"):
    if os.path.isdir(_p) and _p not in sys.path:
        sys.path.append(_p)

import concourse.bass as bass
import concourse.masks as masks
import concourse.mybir as mybir
import concourse.tile as tile
from concourse.bass_utils import run_bass_kernel_spmd

F32 = mybir.dt.float32
F32R = mybir.dt.float32r
BF16 = mybir.dt.bfloat16
AF = mybir.ActivationFunctionType
ALU = mybir.AluOpType

# problem shape (hardcoded per contest contract)
B, F, N, D = 2048, 512, 512, 5
NLEAF = 32
MB, MT = 2, 4                     # batch x tree sharding (MB*MT = 8 cores)
BC, TC = B // MB, N // MT         # 1024, 128 per core
ND = TC * D                       # 640 feature-columns per core
NLO, NHI = 8, 4                   # leaf-id split sizes (lo: depths 0-2, hi: 3-4)
NROW_LO = 3 * TC                  # 384 permuted lo rows (3 tiles)
EPS = 1e-20

_CACHE = {}
LAST = None  # BassKernelResults of the most recent run (for profiling)
FIXUP_WAITS = True  # set False for CoreSim (it can't interp the sem pseudos)
PHASES = 99  # timing-bisect knob: 1=xT+mm1, 2=+gates/ln, 3=+mm2/exp, 4=full
# float32r (single-pass matmul, 1 cy/row vs 4 for fp32): crashes the exec
# unit on TRN2 hardware (NRT_EXEC_UNIT_UNRECOVERABLE) -- keep OFF.
USE_F32R = False
# bf16 matmuls: 1 cy/row vs 4 for fp32 on PE; rel-err budget 2e-2 absorbs it.
USE_BF16 = True
MMDT = BF16 if USE_BF16 else F32
GATES_ON_GPSIMD = False  # gpsimd tensor_scalar measured ~120us per [128,1024] op
# "mono": centered-monomial (Mobius) design -- the leaf product is a
# multilinear polynomial in s_d = 2*gate_d - 1; evaluated with 5 DVE mults +
# a partition-shuffle DMA + the same block-diag matmuls; kills the entire
# Ln/Exp chain (26us of Activation) and mm2 path-sum matmuls.
# "log": the original log-domain design (fallback).
DESIGN = "mono"


def _structure(path_map):
    """Derive path matrices + leaf regroup from the runtime path_map."""
    path = np.asarray(path_map).reshape(NLEAF, D)
    lo_t = [tuple(int(path[l, j]) for j in (0, 1, 2)) for l in range(NLEAF)]
    hi_t = [tuple(int(path[l, j]) for j in (3, 4)) for l in range(NLEAF)]
    lo_ids = sorted(set(lo_t))
    hi_ids = sorted(set(hi_t))
    assert len(lo_ids) <= NLO and len(hi_ids) <= NHI, "path_map does not factor"
    lo_of = {t: i for i, t in enumerate(lo_ids)}
    hi_of = {t: i for i, t in enumerate(hi_ids)}
    P_lo = np.zeros((2 * D, NLO), np.float32)
    for t, i in lo_of.items():
        for e in t:
            P_lo[e, i] += 1.0
    P_hi = np.zeros((2 * D, NHI), np.float32)
    for t, i in hi_of.items():
        for e in t:
            P_hi[e, i] += 1.0
    leaf_hi = np.array([hi_of[t] for t in hi_t], np.int64)
    leaf_lo = np.array([lo_of[t] for t in lo_t], np.int64)
    return P_lo, P_hi, leaf_hi, leaf_lo


def _perm():
    """Permuted nd order: (t,d) d in 0..2 for all trees, then d in 3..4."""
    p = []
    for t in range(TC):
        for d in (0, 1, 2):
            p.append(5 * t + d)
    for t in range(TC):
        for d in (3, 4):
            p.append(5 * t + d)
    return np.array(p, np.int64)


def _mm2_passes(P_lo, P_hi):
    """Host-built lhsT tiles for the path-sum matmuls.

    Returns (pb, sb_passes, sa_passes): pb [NPB,128,128];
    sb_passes[sigma] / sa_passes[alpha] are lists of (pb_idx, tau, sign).
    sign 1 -> rhs Lp, 0 -> rhs Ln.  S row conventions:
      S_lo out-tile sigma: partition 8*tl+lo, trees 16*sigma+tl
      S_hi out-tile alpha: partition 4*tl+hi, trees 32*alpha+tl
    """
    mats, sb_passes, sa_passes = [], [], []
    for sig in range(8):
        passes = []
        for s in (1, 0):
            by_tau = {}
            for tl in range(16):
                t = 16 * sig + tl
                for d in (0, 1, 2):
                    r = 3 * t + d
                    tau, k = r // 128, r % 128
                    m = by_tau.setdefault(tau, np.zeros((128, 128), np.float32))
                    for lo in range(NLO):
                        m[k, 8 * tl + lo] = P_lo[2 * d + s, lo]
            for tau in sorted(by_tau):
                passes.append((len(mats), tau, s))
                mats.append(by_tau[tau])
        sb_passes.append(passes)
    for al in range(4):
        passes = []
        for s in (1, 0):
            by_tau = {}
            for tl in range(32):
                t = 32 * al + tl
                for d in (3, 4):
                    r = NROW_LO + 2 * t + (d - 3)
                    tau, k = r // 128, r % 128
                    m = by_tau.setdefault(tau, np.zeros((128, 128), np.float32))
                    for hi in range(NHI):
                        m[k, 4 * tl + hi] = P_hi[2 * d + s, hi]
            for tau in sorted(by_tau):
                passes.append((len(mats), tau, s))
                mats.append(by_tau[tau])
        sa_passes.append(passes)
    return np.stack(mats), sb_passes, sa_passes


def _blob_offsets(npb):
    """Column offsets (in f32 units) of each region in the single packed
    input blob [128, W].  bf16 regions are stored as f32 column pairs and
    bitcast at DMA time.  Keeping ONE input tensor matters: per-iteration
    launch overhead through the PJRT tunnel scales with input-tensor count
    (~2ms each), dwarfing device time."""
    off, lay = 0, {}
    for name, cols in (("fa", 4 * ND), ("x", 8 * F // 2),
                       ("pb", npb * 64), ("r2l", 512), ("onesd", 256),
                       ("c0", 5), ("c1", 5)):
        lay[name] = off
        off += cols
    return lay, off


def _mobius_C(resp, path_map):
    """Centered-basis Mobius coefficients C[t, jhi(4), jlo(8)]:
    out = sum_{jhi,jlo} C * q_jhi * m_jlo, with monomials of s_d = 2 p_d - 1
    (m: depths 0-2, jlo bit d set -> s_d factor; q: depths 3-4).
    Centering keeps |C| small so bf16 rounding of the monomials is not
    amplified by cancellation (validated: 2.8e-3 vs 1.7e-2 uncentered)."""
    path = np.asarray(path_map).reshape(NLEAF, D)
    assert np.all(path // 2 == np.arange(D)[None, :]), "non-oblivious path_map"
    bits = path & 1
    R = np.asarray(resp, np.float64).reshape(N, NLEAF)
    T = np.zeros((N, 2, 2, 2, 2, 2))
    for l in range(NLEAF):
        b = bits[l]
        T[:, b[0], b[1], b[2], b[3], b[4]] += R[:, l]
    for ax in range(1, 6):
        i0 = [slice(None)] * 6
        i1 = [slice(None)] * 6
        i0[ax], i1[ax] = 0, 1
        a0, a1 = T[tuple(i0)].copy(), T[tuple(i1)].copy()
        T[tuple(i0)] = 0.5 * (a0 + a1)
        T[tuple(i1)] = 0.5 * (a1 - a0)
    # axes t, b0..b4 -> [t, jhi=2*b4+b3, jlo=4*b2+2*b1+b0]
    return np.transpose(T, (0, 5, 4, 3, 2, 1)).reshape(N, 4, 8)


def _blob_offsets_mono():
    """Single packed input [128, W] (f32 cols; bf16 regions as col pairs).
    w = host-softmaxed choice weights (d-major), xt = host-transposed x."""
    off, lay = 0, {}
    for name, cols in (("w", 4 * ND // 2), ("xt", 4 * BC // 2),
                       ("cm", 512), ("onesd", 256), ("a", 5), ("c0", 5)):
        lay[name] = off
        off += cols
    return lay, off


def _build_nc_mono():
    nc = bass.Bass()
    lay, W = _blob_offsets_mono()
    blob = nc.dram_tensor("blob", [128, W], F32, kind="ExternalInput")
    out_d = nc.dram_tensor("out_t", [TC, BC], F32, kind="ExternalOutput")
    deps = []

    with tile.TileContext(nc) as tc:
        with (
            tc.tile_pool(name="const", bufs=1) as cpool,
            tc.tile_pool(name="big", bufs=1) as bpool,
            tc.tile_pool(name="work", bufs=2) as wpool,
            tc.tile_pool(name="out", bufs=1) as opool,
            tc.tile_pool(name="psum", bufs=3, space="PSUM") as pp,
            tc.tile_pool(name="psink", bufs=1, space="PSUM") as psink,
        ):
            sink = psink.tile([1, 1], F32, tag="sink")

            def pe_touch(ap):
                return nc.tensor.matmul(sink[:], ap, ap, start=True, stop=True,
                                        skip_group_check=True)

            def breg(name, cols):
                return blob[:, lay[name]:lay[name] + cols]

            # ---- inputs: spread across the two HWDGE queues (SP + Act) ----
            onest = cpool.tile([128, 4 * 128], MMDT, tag="ones")
            nc.sync.dma_start(onest[:], breg("onesd", 256).bitcast(MMDT))
            t_on = pe_touch(onest[:, 0:1])
            cmt = cpool.tile([128, 8 * 128], MMDT, tag="cm")
            nc.sync.dma_start(cmt[:], breg("cm", 512).bitcast(MMDT))
            t_cm = pe_touch(cmt[:, 0:1])
            a_sb = cpool.tile([128, 5], F32, tag="a_sb")
            nc.sync.dma_start(a_sb[:], breg("a", 5))
            c0t = cpool.tile([128, 5], F32, tag="c0")
            nc.sync.dma_start(c0t[:], breg("c0", 5))

            wt = bpool.tile([128, 4 * ND], MMDT, tag="Wt")
            w0 = lay["w"]
            t_wt = []
            for ft in range(4):
                nc.scalar.dma_start(
                    wt[:, ND * ft:ND * (ft + 1)],
                    blob[:, w0 + 320 * ft:w0 + 320 * (ft + 1)].bitcast(MMDT))
                t_wt.append(pe_touch(wt[:, ND * ft:ND * ft + 1]))
            xts = bpool.tile([128, 4 * BC], MMDT, tag="xT")
            x0 = lay["xt"]
            t_xt = []
            for ft in range(4):
                nc.sync.dma_start(
                    xts[:, BC * ft:BC * (ft + 1)],
                    blob[:, x0 + 512 * ft:x0 + 512 * (ft + 1)].bitcast(MMDT))
                t_xt.append(pe_touch(xts[:, BC * ft:BC * ft + 1]))

            # ---- mm1 + gates: s_d = clip(a*FV + c0, -1, 1) ----
            # M [128 trees, 8*BC] bf16: lo-monomial block k (k bit d -> s_d);
            # H [128 trees, 4*BC] bf16: hi block j (1, s3, s4, s3*s4).
            # Depths 3,4 first so the H shuffle can overlap lo-depth mm1.
            M = bpool.tile([128, 8 * BC], MMDT, tag="M")
            H = bpool.tile([128, 4 * BC], MMDT, tag="H")
            nc.vector.memset(M[:, 0:BC], 1.0)
            nc.vector.memset(H[:, 0:BC], 1.0)
            kcol = {0: 1, 1: 2, 2: 4}   # depth -> lo-monomial block

            def mm1_gate(d):
                fv = pp.tile([128, BC], F32, tag="big", name=f"fv{d}")
                for ft in range(4):
                    st, sp = ft == 0, ft == 3
                    for h in range(2):
                        mm = nc.tensor.matmul(
                            fv[:, bass.ts(h, 512)],
                            wt[:, ND * ft + 128 * d:ND * ft + 128 * (d + 1)],
                            xts[:, BC * ft + 512 * h:BC * ft + 512 * (h + 1)],
                            start=st, stop=sp)
                        deps.extend([(mm, t_wt[ft]), (mm, t_xt[ft])])
                ut = wpool.tile([128, BC], F32, tag="u", name=f"u{d}")
                nc.vector.tensor_scalar(ut[:], fv[:], a_sb[:, d:d + 1],
                                        c0t[:, d:d + 1], ALU.mult, ALU.add)
                if d < 3:
                    dst = M[:, BC * kcol[d]:BC * (kcol[d] + 1)]
                else:
                    dst = H[:, BC * (d - 2):BC * (d - 1)]
                nc.vector.tensor_scalar(dst, ut[:], 1.0, -1.0,
                                        ALU.min, ALU.max)

            def mblk(k):
                return M[:, BC * k:BC * (k + 1)]

            mm1_gate(3)
            mm1_gate(4)
            nc.vector.tensor_mul(H[:, 3 * BC:4 * BC], H[:, BC:2 * BC],
                                 H[:, 2 * BC:3 * BC])
            qsh = []
            for al in range(4):
                qt = bpool.tile([128, BC], MMDT, tag=f"qsh{al}")
                eng = nc.scalar if al % 2 else nc.sync
                eng.dma_start(qt[:], H[32 * al:32 * (al + 1), :])
                qsh.append(qt)
            for d in (0, 1, 2):
                mm1_gate(d)
            nc.vector.tensor_mul(mblk(3), mblk(1), mblk(2))
            nc.vector.tensor_mul(mblk(5), mblk(1), mblk(4))
            nc.vector.tensor_mul(mblk(6), mblk(2), mblk(4))
            nc.vector.tensor_mul(mblk(7), mblk(3), mblk(4))

            # ---- partition shuffle: tree-major -> block layout ----
            # Msh[sg][8*tl + k, b] = M[16*sg + tl, BC*k + b]  (flat row-major
            # copy of a [16, 8*BC] slice into [128, BC]); alternate the two
            # HWDGE queues so the 8 copies run in parallel pairs.
            msh = []
            t_msh = []
            for sg in range(8):
                mt = bpool.tile([128, BC], MMDT, tag=f"msh{sg}")
                eng = nc.scalar if sg % 2 else nc.sync
                eng.dma_start(mt[:], M[16 * sg:16 * (sg + 1), :])
                msh.append(mt)
                t_msh.append(pe_touch(mt[:, 0:1]))

            # ---- z = C . m (block-diag), P = Q * z, ones-reduce ----
            # PE emission order pipelines z(al+1) ahead of mm4(al) so PE
            # never stalls on the DVE P-multiply.
            outp = pp.tile([128, BC], F32, tag="big")
            zs = {}

            def emit_z(al):
                z = pp.tile([128, BC], F32, tag="big", name=f"z{al}")
                for j in range(2):
                    sg = 2 * al + j
                    for h in range(2):
                        mm = nc.tensor.matmul(
                            z[:, bass.ts(h, 512)],
                            cmt[:, bass.ts(sg, 128)],
                            msh[sg][:, bass.ts(h, 512)],
                            start=j == 0, stop=j == 1)
                        deps.extend([(mm, t_cm), (mm, t_msh[sg])])
                zs[al] = z

            emit_z(0)
            emit_z(1)
            for al in range(4):
                pt = wpool.tile([128, BC], MMDT, tag="P", name=f"pt{al}")
                nc.vector.tensor_mul(pt[:], qsh[al][:], zs[al][:])
                t_pt = pe_touch(pt[:, 0:1])
                if al + 2 <= 3:
                    emit_z(al + 2)
                for h in range(2):
                    mm = nc.tensor.matmul(
                        outp[:, bass.ts(h, 512)],
                        onest[:, bass.ts(al, 128)],
                        pt[:, bass.ts(h, 512)],
                        start=al == 0, stop=al == 3,
                        skip_group_check=True)
                    deps += [(mm, t_on), (mm, t_pt)]
            out_sb = opool.tile([128, BC], F32, tag="osb")
            nc.vector.tensor_copy(out_sb[:], outp[:])
            nc.sync.dma_start(out_d[:], out_sb[:])

            from concourse.tile import add_dep_helper
            for a, b in deps:
                add_dep_helper(a.ins, b.ins, sync=False,
                               reason="PE pre-sync absorbs extra waits")
    if FIXUP_WAITS:
        _split_excess_waits(nc)
    return nc


def _build_nc(npb, sb_passes, sa_passes):
    MMDT = F32R if USE_F32R else (BF16 if USE_BF16 else F32)
    lay, W = _blob_offsets(npb)
    nc = bass.Bass()
    blob = nc.dram_tensor("blob", [128, W], F32, kind="ExternalInput")

    def reg(name, cols, dt=F32):
        sl = blob[:, lay[name]:lay[name] + cols]
        return sl.bitcast(dt) if dt != F32 else sl

    out_d = nc.dram_tensor("out_t", [TC, BC], F32, kind="ExternalOutput")

    deps = []  # (dependent BassInstruction, dependency BassInstruction)

    with tile.TileContext(nc) as tc:
        with (
            tc.tile_pool(name="const", bufs=1) as cpool,
            tc.tile_pool(name="big", bufs=1) as bpool,
            tc.tile_pool(name="work", bufs=2) as wpool,
            tc.tile_pool(name="out", bufs=1) as opool,
            tc.tile_pool(name="psum", bufs=3, space="PSUM") as pp,
            tc.tile_pool(name="psink", bufs=1, space="PSUM") as psink,
        ):
            # PE matmuls (incl. transpose-mode) only tolerate ONE sync wait
            # after walrus lowering; "touch" matmuls absorb producer-engine
            # waits into PE's vector clock ahead of the real matmuls.
            sink = psink.tile([1, 1], F32, tag="sink")

            def pe_touch(ap):
                if ap.dtype == F32R:
                    ap = ap.bitcast(F32)
                return nc.tensor.matmul(sink[:], ap, ap, start=True, stop=True,
                                        skip_group_check=True)

            def mmop(ap):
                return ap.bitcast(F32) if MMDT == F32R else ap

            # ---- constants (one DMA each) ----
            idt = cpool.tile([128, 128], F32, tag="idt")
            masks.make_identity(nc, idt[:])
            t_idt = pe_touch(idt[:, 0:1])
            if MMDT == BF16:
                idm = cpool.tile([128, 128], MMDT, tag="idm")
                nc.vector.tensor_copy(idm[:], idt[:])
                t_idm = pe_touch(idm[:, 0:1])
            else:
                idm, t_idm = idt, t_idt
            assert MMDT == BF16, "blob layout assumes bf16 matmul dtype"
            onest = cpool.tile([128, 4 * 128], MMDT, tag="ones")
            nc.sync.dma_start(onest[:], reg("onesd", 256, MMDT))
            t_on = pe_touch(onest[:, 0:1])
            pbt = cpool.tile([128, npb * 128], MMDT, tag="pb")
            nc.sync.dma_start(pbt[:], reg("pb", npb * 64, MMDT))
            t_pb = pe_touch(pbt[:, 0:1])
            r2t = cpool.tile([128, 8 * 128], MMDT, tag="r2")
            nc.sync.dma_start(r2t[:], reg("r2l", 512, MMDT))
            t_r2 = pe_touch(r2t[:, 0:1])
            c0t = cpool.tile([128, 5], F32, tag="c0")
            nc.sync.dma_start(c0t[:], reg("c0", 5))
            c1t = cpool.tile([128, 5], F32, tag="c1")
            nc.sync.dma_start(c1t[:], reg("c1", 5))
            ones_col = cpool.tile([128, 1], MMDT, tag="onescol")
            ones_raw = cpool.tile([128, 1], F32, tag="onescolr")
            nc.vector.memset(ones_raw[:], 1.0)
            nc.vector.tensor_copy(ones_col[:], ones_raw[:])
            t_oc = pe_touch(ones_col[:])

            # ---- E = exp(A), column sums, a = c1/colsum ----
            et = bpool.tile([128, 4 * ND], MMDT, tag="E")
            fa0 = lay["fa"]
            for ft in range(4):
                araw = wpool.tile([128, ND], F32, tag="u")
                nc.sync.dma_start(araw[:],
                                  blob[:, fa0 + ND * ft:fa0 + ND * (ft + 1)])
                nc.scalar.activation(et[:, ND * ft:ND * (ft + 1)], araw[:], AF.Exp)
            cs_a = pp.tile([1, 512], F32, tag="big")
            cs_b = pp.tile([1, 128], F32, tag="big")
            for ft in range(4):
                st, sp = ft == 0, ft == 3
                m1 = nc.tensor.matmul(cs_a[:], mmop(ones_col[:]),
                                      mmop(et[:, ND * ft:ND * ft + 512]),
                                      start=st, stop=sp)
                m2 = nc.tensor.matmul(cs_b[:], mmop(ones_col[:]),
                                      mmop(et[:, ND * ft + 512:ND * (ft + 1)]),
                                      start=st, stop=sp)
                deps += [(m1, t_oc), (m2, t_oc)]
            invs = cpool.tile([1, ND], F32, tag="invs")
            nc.vector.reciprocal(invs[:, 0:512], cs_a[:])
            nc.vector.reciprocal(invs[:, 512:640], cs_b[:])
            a_sb = cpool.tile([128, 5], F32, tag="a_sb")
            for t in range(5):
                tp = pp.tile([128, 128], F32, tag="big")
                tr = nc.tensor.transpose(tp[:, 0:1], invs[0:1, bass.ts(t, 128)],
                                         idt[0:1, 0:1])
                deps.append((tr, t_idt))
                nc.vector.tensor_scalar(a_sb[:, t:t + 1], tp[:, 0:1],
                                        c1t[:, t:t + 1], None, ALU.mult)

            # ---- x^T via PE transpose ----
            xts = bpool.tile([128, 4 * BC], MMDT, tag="xT")
            xraw = bpool.tile([128, 8 * F], MMDT, tag="xraw")
            x0 = lay["x"]
            for bt in range(8 if PHASES >= 1 else 0):
                nc.sync.dma_start(
                    xraw[:, F * bt:F * (bt + 1)],
                    blob[:, x0 + 256 * bt:x0 + 256 * (bt + 1)].bitcast(MMDT))
            for ft in range(4 if PHASES >= 1 else 0):
                tp = pp.tile([128, BC], MMDT, tag="big")
                for bt in range(8):
                    tr = nc.tensor.transpose(
                        tp[:, bass.ts(bt, 128)],
                        xraw[:, F * bt + 128 * ft:F * bt + 128 * (ft + 1)],
                        idm[:])
                    deps.append((tr, t_idm))
                nc.vector.tensor_copy(xts[:, BC * ft:BC * (ft + 1)], tp[:])

            # ---- mm1 + gates + logs, per nd-tile ----
            lpt = bpool.tile([128, 5 * BC], MMDT, tag="Lp")
            lnt = bpool.tile([128, 5 * BC], MMDT, tag="Ln")
            t_u = []
            last_gate = None
            for t in range(5 if PHASES >= 1 else 0):
                fv = pp.tile([128, BC], F32, tag="big")
                for ft in range(4):
                    st, sp = ft == 0, ft == 3
                    for h in range(2):
                        nc.tensor.matmul(
                            fv[:, bass.ts(h, 512)],
                            et[:, ND * ft + 128 * t:ND * ft + 128 * (t + 1)],
                            xts[:, BC * ft + 512 * h:BC * ft + 512 * (h + 1)],
                            start=st, stop=sp)
                ut = wpool.tile([128, BC], F32, tag="u")
                nc.vector.tensor_scalar(ut[:], fv[:], a_sb[:, t:t + 1],
                                        c0t[:, t:t + 1], ALU.mult, ALU.add)
                t_u.append(pe_touch(ut[:, 0:1]))
                geng = nc.gpsimd if GATES_ON_GPSIMD else nc.vector
                upt = wpool.tile([128, BC], F32, tag="up")
                geng.tensor_scalar(upt[:], ut[:], 1.0, EPS, ALU.min, ALU.max)
                unt = wpool.tile([128, BC], F32, tag="un")
                geng.tensor_scalar(unt[:], ut[:], -1.0, 1.0,
                                   ALU.mult, ALU.add)
                geng.tensor_scalar(unt[:], unt[:], 1.0, EPS,
                                   ALU.min, ALU.max)
                if PHASES >= 2:
                    nc.scalar.activation(lpt[:, BC * t:BC * (t + 1)],
                                         upt[:], AF.Ln)
                    nc.scalar.activation(lnt[:, BC * t:BC * (t + 1)],
                                         unt[:], AF.Ln)
                last_gate = unt

            if PHASES < 3:
                out_sb = opool.tile([128, BC], F32, tag="osb")
                src_t = last_gate if PHASES < 2 else lpt
                if src_t is None:
                    nc.vector.memset(out_sb[:], 0.0)
                elif src_t is lpt:
                    nc.vector.tensor_copy(out_sb[:], src_t[:, 0:BC])
                else:
                    nc.vector.tensor_copy(out_sb[:], src_t[:])
                nc.sync.dma_start(out_d[:], out_sb[:])
                from concourse.tile import add_dep_helper as _adh
                for a, b in deps:
                    _adh(a.ins, b.ins, sync=False, reason="pre-sync")
                deps.clear()
            if PHASES < 3:
                pass  # skip back half entirely

            def lsrc(s, tau):
                src = lpt if s == 1 else lnt
                return src[:, BC * tau:BC * (tau + 1)]

            # ---- mm2 (path sums) + exp ----
            ebt = bpool.tile([128, 8 * BC], MMDT, tag="eB")
            for sg in range(8 if PHASES >= 3 else 0):
                sb = pp.tile([128, BC], F32, tag="big")
                passes = sb_passes[sg]
                for i, (pi, tau, s) in enumerate(passes):
                    st, sp = i == 0, i == len(passes) - 1
                    for h in range(2):
                        mm = nc.tensor.matmul(
                            sb[:, bass.ts(h, 512)],
                            pbt[:, bass.ts(pi, 128)],
                            lsrc(s, tau)[:, bass.ts(h, 512)],
                            start=st, stop=sp)
                        deps += [(mm, t_pb)] + [(mm, tu) for tu in t_u]
                nc.scalar.activation(ebt[:, BC * sg:BC * (sg + 1)], sb[:], AF.Exp)
            eat = bpool.tile([128, 4 * BC], F32, tag="eA")
            for al in range(4 if PHASES >= 3 else 0):
                sa = pp.tile([128, BC], F32, tag="big")
                passes = sa_passes[al]
                for i, (pi, tau, s) in enumerate(passes):
                    st, sp = i == 0, i == len(passes) - 1
                    for h in range(2):
                        mm = nc.tensor.matmul(
                            sa[:, bass.ts(h, 512)],
                            pbt[:, bass.ts(pi, 128)],
                            lsrc(s, tau)[:, bass.ts(h, 512)],
                            start=st, stop=sp)
                        deps += [(mm, t_pb)] + [(mm, tu) for tu in t_u]
                nc.scalar.activation(eat[:, BC * al:BC * (al + 1)], sa[:], AF.Exp)

            # ---- mm3 (z = R2 . e_lo), P = e_hi * z, mm4 (ones-reduce) ----
            if PHASES == 3:
                out_sb = opool.tile([128, BC], F32, tag="osb")
                nc.vector.tensor_copy(out_sb[:], ebt[:, 0:BC])
                nc.sync.dma_start(out_d[:], out_sb[:])
            outp = pp.tile([128, BC], F32, tag="big")
            t_prev = None
            for al in range(4 if PHASES >= 4 else 0):
                z = pp.tile([128, BC], F32, tag="big")
                for j in range(2):
                    sg = 2 * al + j
                    for h in range(2):
                        mm = nc.tensor.matmul(
                            z[:, bass.ts(h, 512)],
                            r2t[:, bass.ts(sg, 128)],
                            ebt[:, BC * sg + 512 * h:BC * sg + 512 * (h + 1)],
                            start=j == 0, stop=j == 1)
                        deps.append((mm, t_r2))
                        if t_prev is not None:
                            deps.append((mm, t_prev))
                pt = wpool.tile([128, BC], MMDT, tag="P")
                nc.vector.tensor_mul(pt[:], eat[:, BC * al:BC * (al + 1)], z[:])
                t_pt = pe_touch(pt[:, 0:1])
                for h in range(2):
                    mm = nc.tensor.matmul(
                        outp[:, bass.ts(h, 512)],
                        onest[:, bass.ts(al, 128)],
                        pt[:, bass.ts(h, 512)],
                        start=al == 0, stop=al == 3,
                        skip_group_check=True)
                    deps += [(mm, t_on), (mm, t_pt)]
                t_prev = t_pt
            if PHASES >= 4:
                out_sb = opool.tile([128, BC], F32, tag="osb")
                nc.vector.tensor_copy(out_sb[:], outp[:])
                nc.sync.dma_start(out_d[:], out_sb[:])

            from concourse.tile import add_dep_helper
            for a, b in deps:
                add_dep_helper(a.ins, b.ins, sync=False,
                               reason="PE pre-sync absorbs extra waits")
    if FIXUP_WAITS:
        _split_excess_waits(nc)
    return nc


def _split_excess_waits(nc):
    """Walrus codegen only fits ONE sync wait on PE Matmult and DMACopy
    instructions ("Too many sync wait commands").  Hoist the extras onto
    preceding same-engine InstEventSemaphore pseudos (one wait each), which
    the sequencer executes before the limited instruction."""
    exempt = {"InstEventSemaphore", "InstUnconditionalBranch",
              "InstISA", "InstHalt"}
    nfix = 0
    for fn in nc.m.functions:
        for bb in fn.blocks:
            il = bb.instructions
            out = []
            for inst in il:
                si = inst.sync_info
                lim = None if type(inst).__name__ in exempt else 1
                if si is not None and lim is not None and len(si.on_wait) > lim:
                    keep = list(si.on_wait[-lim:])
                    for w in si.on_wait[:-lim]:
                        nfix += 1
                        ev = mybir.InstEventSemaphore(
                            name=f"I-waitfix-{nfix}",
                            engine=inst.engine,
                            ins=[], outs=[],
                            sync_info=mybir.SyncInfo(on_wait=[w], on_update=[]),
                        )
                        ev.bass_nofuse = True
                        out.append(ev)
                    inst.sync_info = mybir.SyncInfo(
                        on_wait=keep, on_update=list(si.on_update))
                out.append(inst)
            il[:] = out
            assert len(bb.instructions) == len(out)
    return nfix


def _prep(path_map):
    key = (np.asarray(path_map).tobytes(), PHASES, GATES_ON_GPSIMD, USE_F32R,
           USE_BF16, DESIGN)
    if key not in _CACHE:
        if DESIGN == "mono":
            _CACHE[key] = (None, None, None, _build_nc_mono())
        else:
            P_lo, P_hi, leaf_hi, leaf_lo = _structure(path_map)
            pb, sb_passes, sa_passes = _mm2_passes(P_lo, P_hi)
            nc = _build_nc(pb.shape[0], sb_passes, sa_passes)
            _CACHE[key] = (pb, leaf_hi, leaf_lo, nc)
    return _CACHE[key]


def _onesd_pack():
    onesd = np.zeros((4, 128, 128), np.float32)
    for al in range(4):
        for tl in range(32):
            for hi in range(NHI):
                onesd[al, 4 * tl + hi, 32 * al + tl] = 1.0
    return np.ascontiguousarray(onesd.transpose(1, 0, 2).reshape(128, 512))


def _build_in_maps_mono(x, fa, thr, lt, resp, path_map, nc):
    mmnp = mybir.dt.np(BF16)
    lay, W = _blob_offsets_mono()
    fa64 = np.asarray(fa, np.float64)
    E = np.exp(fa64 - fa64.max(axis=0, keepdims=True))
    cw = (E / E.sum(axis=0, keepdims=True)).astype(np.float32)  # [F, N*D]
    invtemp = np.exp(-np.asarray(lt, np.float64))
    a_all = invtemp.astype(np.float32)                      # [N, D]
    c0_all = (-np.asarray(thr, np.float64) * invtemp).astype(np.float32)
    C = _mobius_C(resp, path_map)                           # [N, 4, 8]
    onesd = _onesd_pack()

    def asf32pairs(a):
        a = np.ascontiguousarray(a.astype(mmnp))
        return a.view(np.float32)

    in_maps = []
    for c in range(8):
        bi, ti = c // MT, c % MT
        t0 = ti * TC
        # d-major column order: tile d holds all TC trees at depth d
        cols = np.array([(t0 + t) * D + d for d in range(D)
                         for t in range(TC)])
        wc = np.ascontiguousarray(cw[:, cols])
        blob = np.zeros((128, W), np.float32)
        for ft in range(4):
            blob[:, lay["w"] + 320 * ft:lay["w"] + 320 * (ft + 1)] = \
                asf32pairs(wc[128 * ft:128 * (ft + 1)])
        xT = np.ascontiguousarray(x[bi * BC:(bi + 1) * BC].T)   # [F, BC]
        for ft in range(4):
            blob[:, lay["xt"] + 512 * ft:lay["xt"] + 512 * (ft + 1)] = \
                asf32pairs(xT[128 * ft:128 * (ft + 1)])
        cm8 = np.zeros((8, 128, 128), np.float32)
        for sg in range(8):
            off = 64 * (sg % 2)
            for tl in range(16):
                t = t0 + 16 * sg + tl
                for k in range(8):
                    for j in range(4):
                        cm8[sg, 8 * tl + k, off + 4 * tl + j] = C[t, j, k]
        cm2 = np.ascontiguousarray(cm8.transpose(1, 0, 2).reshape(128, 1024))
        blob[:, lay["cm"]:lay["cm"] + 512] = asf32pairs(cm2)
        blob[:, lay["onesd"]:lay["onesd"] + 256] = asf32pairs(onesd)
        blob[:, lay["a"]:lay["a"] + 5] = a_all[t0:t0 + TC]
        blob[:, lay["c0"]:lay["c0"] + 5] = c0_all[t0:t0 + TC]
        in_maps.append({"blob": blob})
    return in_maps, nc


def build_in_maps(x, feat_attention, feature_thresholds, log_temperatures,
                  response, path_map):
    x = np.ascontiguousarray(np.asarray(x, np.float32))
    fa = np.asarray(feat_attention, np.float32)
    thr = np.asarray(feature_thresholds, np.float32)
    lt = np.asarray(log_temperatures, np.float32)
    resp = np.asarray(response, np.float32).reshape(N, NLEAF)
    pb, leaf_hi, leaf_lo, nc = _prep(path_map)
    if DESIGN == "mono":
        return _build_in_maps_mono(x, fa, thr, lt, resp, path_map, nc)

    perm = _perm()
    invtemp = np.exp(-lt)                        # [N, D] host weight prep
    c1_all = (0.5 * invtemp).reshape(N * D)
    c0_all = (0.5 - 0.5 * thr * invtemp).reshape(N * D)
    # R2[n, hi, lo] = sum of response over leaves in that (hi, lo) group
    R2 = np.zeros((N, NHI, NLO), np.float32)
    np.add.at(R2, (slice(None), leaf_hi, leaf_lo), resp)

    onesd = np.zeros((4, 128, 128), np.float32)
    for al in range(4):
        for tl in range(32):
            for hi in range(NHI):
                onesd[al, 4 * tl + hi, 32 * al + tl] = 1.0
    onesd = np.ascontiguousarray(onesd.transpose(1, 0, 2).reshape(128, 512))
    pb2 = np.ascontiguousarray(
        pb.transpose(1, 0, 2).reshape(128, pb.shape[0] * 128))

    in_maps = []
    for c in range(8):
        bi, ti = c // MT, c % MT
        t0 = ti * TC
        cols = t0 * D + perm                      # permuted global nd columns
        c0 = np.ascontiguousarray(c0_all[cols].reshape(5, 128).T)
        c1 = np.ascontiguousarray(c1_all[cols].reshape(5, 128).T)
        r2l = np.zeros((8, 128, 128), np.float32)
        for sg in range(8):
            off = 64 * (sg % 2)
            for tl in range(16):
                t = t0 + 16 * sg + tl
                for hi in range(NHI):
                    for lo in range(NLO):
                        r2l[sg, 8 * tl + lo, off + 4 * tl + hi] = R2[t, hi, lo]
        r2l = np.ascontiguousarray(r2l.transpose(1, 0, 2).reshape(128, 1024))
        mmnp = mybir.dt.np(BF16)
        lay, W = _blob_offsets(pb.shape[0])

        def asf32pairs(a):
            """bf16 [128, 2k] -> f32-viewed [128, k] for blob packing."""
            a = np.ascontiguousarray(a.astype(mmnp))
            return a.view(np.float32)

        blob = np.zeros((128, W), np.float32)
        fac = np.ascontiguousarray(fa[:, t0 * D:(t0 + TC) * D][:, perm])
        for ft in range(4):
            blob[:, lay["fa"] + ND * ft:lay["fa"] + ND * (ft + 1)] = \
                fac[128 * ft:128 * (ft + 1)]
        xc = x[bi * BC:(bi + 1) * BC]
        for bt in range(8):
            blob[:, lay["x"] + 256 * bt:lay["x"] + 256 * (bt + 1)] = \
                asf32pairs(xc[128 * bt:128 * (bt + 1)])
        blob[:, lay["pb"]:lay["pb"] + pb.shape[0] * 64] = asf32pairs(pb2)
        blob[:, lay["r2l"]:lay["r2l"] + 512] = asf32pairs(r2l)
        blob[:, lay["onesd"]:lay["onesd"] + 256] = asf32pairs(onesd)
        blob[:, lay["c0"]:lay["c0"] + 5] = c0
        blob[:, lay["c1"]:lay["c1"] + 5] = c1
        in_maps.append({"blob": blob})
    return in_maps, nc


def kernel(x, feat_attention, feature_thresholds, log_temperatures,
           response, path_map):
    in_maps, nc = build_in_maps(x, feat_attention, feature_thresholds,
                                log_temperatures, response, path_map)
    res = run_bass_kernel_spmd(nc, in_maps, list(range(8)))
    global LAST
    LAST = res
    out = np.empty((B, N), np.float32)
    for c in range(8):
        bi, ti = c // MT, c % MT
        out[bi * BC:(bi + 1) * BC, ti * TC:(ti + 1) * TC] = res.results[c]["out_t"].T
    return out



# revision 30
# speedup vs baseline: 320.6720x; 1.0620x over previous
"""Trainium2 Bass kernel for nn_DeTree (oblivious decision-tree / MoE routing).

Full-input contract: kernel(**inputs) takes the unsharded inputs and returns
the full [2048, 512] output.  Internally shards 2-way over batch x 4-way over
trees across 8 NeuronCores (SPMD, no collectives), runs a Tile/Bass kernel,
and reassembles on host.

Math (per core, B=1024 batch rows, T=128 trees, nd=640 feature-columns):
  E = exp(feat_attention)                 (softmax numerator; denominator is
                                           folded into a per-row affine)
  FV^T = E^T x^T                          (mm1, PE; x transposed on-chip)
  u    = a*FV + c0   (= sparsemoid pre-clip logit 0.5 + 0.5*tl)
  Lp   = ln(clip(u, eps, 1)), Ln = ln(clip(1-u, eps, 1))
  S_lo/S_hi = path sums of logs           (mm2, PE; 0/1 path matrices)
  e_lo = exp(S_lo) [8/tree], e_hi = exp(S_hi) [4/tree]
  z    = R2 . e_lo                        (mm3, PE; response folded in)
  out  = sum_hi e_hi * z                  (DVE mul + mm4 ones-reduce)
The leaf product over depth 5 is exp(sum of logs); leaves are split into
(lo: depths 0-2 -> 8 ids) x (hi: depths 3,4 -> 4 ids) so only 12 exps/tree
are needed instead of 32.  clip eps=1e-20 makes exp underflow to exactly 0
where the reference gate is exactly 0.
"""

import os
import sys

import numpy as np

for _p in ("/opt/trn_rl_repo", "/root/.axon_site/_ro/trn_rl_repo"):
    if os.path.isdir(_p) and _p not in sys.path:
        sys.path.append(_p)

import concourse.bass as bass
import concourse.masks as masks
import concourse.mybir as mybir
import concourse.tile as tile
from concourse.bass_utils import run_bass_kernel_spmd

F32 = mybir.dt.float32
F32R = mybir.dt.float32r
BF16 = mybir.dt.bfloat16
AF = mybir.ActivationFunctionType
ALU = mybir.AluOpType

# problem shape (hardcoded per contest contract)
B, F, N, D = 2048, 512, 512, 5
NLEAF = 32
MB, MT = 2, 4                     # batch x tree sharding (MB*MT = 8 cores)
BC, TC = B // MB, N // MT         # 1024, 128 per core
ND = TC * D                       # 640 feature-columns per core
NLO, NHI = 8, 4                   # leaf-id split sizes (lo: depths 0-2, hi: 3-4)
NROW_LO = 3 * TC                  # 384 permuted lo rows (3 tiles)
EPS = 1e-20

_CACHE = {}
LAST = None  # BassKernelResults of the most recent run (for profiling)
FIXUP_WAITS = True  # set False for CoreSim (it can't interp the sem pseudos)
PHASES = 99  # timing-bisect knob: 1=xT+mm1, 2=+gates/ln, 3=+mm2/exp, 4=full
# float32r (single-pass matmul, 1 cy/row vs 4 for fp32): crashes the exec
# unit on TRN2 hardware (NRT_EXEC_UNIT_UNRECOVERABLE) -- keep OFF.
USE_F32R = False
# bf16 matmuls: 1 cy/row vs 4 for fp32 on PE; rel-err budget 2e-2 absorbs it.
USE_BF16 = True
MMDT = BF16 if USE_BF16 else F32
GATES_ON_GPSIMD = False  # gpsimd tensor_scalar measured ~120us per [128,1024] op
# "mono": centered-monomial (Mobius) design -- the leaf product is a
# multilinear polynomial in s_d = 2*gate_d - 1; evaluated with 5 DVE mults +
# a partition-shuffle DMA + the same block-diag matmuls; kills the entire
# Ln/Exp chain (26us of Activation) and mm2 path-sum matmuls.
# "log": the original log-domain design (fallback).
DESIGN = "mono"


def _structure(path_map):
    """Derive path matrices + leaf regroup from the runtime path_map."""
    path = np.asarray(path_map).reshape(NLEAF, D)
    lo_t = [tuple(int(path[l, j]) for j in (0, 1, 2)) for l in range(NLEAF)]
    hi_t = [tuple(int(path[l, j]) for j in (3, 4)) for l in range(NLEAF)]
    lo_ids = sorted(set(lo_t))
    hi_ids = sorted(set(hi_t))
    assert len(lo_ids) <= NLO and len(hi_ids) <= NHI, "path_map does not factor"
    lo_of = {t: i for i, t in enumerate(lo_ids)}
    hi_of = {t: i for i, t in enumerate(hi_ids)}
    P_lo = np.zeros((2 * D, NLO), np.float32)
    for t, i in lo_of.items():
        for e in t:
            P_lo[e, i] += 1.0
    P_hi = np.zeros((2 * D, NHI), np.float32)
    for t, i in hi_of.items():
        for e in t:
            P_hi[e, i] += 1.0
    leaf_hi = np.array([hi_of[t] for t in hi_t], np.int64)
    leaf_lo = np.array([lo_of[t] for t in lo_t], np.int64)
    return P_lo, P_hi, leaf_hi, leaf_lo


def _perm():
    """Permuted nd order: (t,d) d in 0..2 for all trees, then d in 3..4."""
    p = []
    for t in range(TC):
        for d in (0, 1, 2):
            p.append(5 * t + d)
    for t in range(TC):
        for d in (3, 4):
            p.append(5 * t + d)
    return np.array(p, np.int64)


def _mm2_passes(P_lo, P_hi):
    """Host-built lhsT tiles for the path-sum matmuls.

    Returns (pb, sb_passes, sa_passes): pb [NPB,128,128];
    sb_passes[sigma] / sa_passes[alpha] are lists of (pb_idx, tau, sign).
    sign 1 -> rhs Lp, 0 -> rhs Ln.  S row conventions:
      S_lo out-tile sigma: partition 8*tl+lo, trees 16*sigma+tl
      S_hi out-tile alpha: partition 4*tl+hi, trees 32*alpha+tl
    """
    mats, sb_passes, sa_passes = [], [], []
    for sig in range(8):
        passes = []
        for s in (1, 0):
            by_tau = {}
            for tl in range(16):
                t = 16 * sig + tl
                for d in (0, 1, 2):
                    r = 3 * t + d
                    tau, k = r // 128, r % 128
                    m = by_tau.setdefault(tau, np.zeros((128, 128), np.float32))
                    for lo in range(NLO):
                        m[k, 8 * tl + lo] = P_lo[2 * d + s, lo]
            for tau in sorted(by_tau):
                passes.append((len(mats), tau, s))
                mats.append(by_tau[tau])
        sb_passes.append(passes)
    for al in range(4):
        passes = []
        for s in (1, 0):
            by_tau = {}
            for tl in range(32):
                t = 32 * al + tl
                for d in (3, 4):
                    r = NROW_LO + 2 * t + (d - 3)
                    tau, k = r // 128, r % 128
                    m = by_tau.setdefault(tau, np.zeros((128, 128), np.float32))
                    for hi in range(NHI):
                        m[k, 4 * tl + hi] = P_hi[2 * d + s, hi]
            for tau in sorted(by_tau):
                passes.append((len(mats), tau, s))
                mats.append(by_tau[tau])
        sa_passes.append(passes)
    return np.stack(mats), sb_passes, sa_passes


def _blob_offsets(npb):
    """Column offsets (in f32 units) of each region in the single packed
    input blob [128, W].  bf16 regions are stored as f32 column pairs and
    bitcast at DMA time.  Keeping ONE input tensor matters: per-iteration
    launch overhead through the PJRT tunnel scales with input-tensor count
    (~2ms each), dwarfing device time."""
    off, lay = 0, {}
    for name, cols in (("fa", 4 * ND), ("x", 8 * F // 2),
                       ("pb", npb * 64), ("r2l", 512), ("onesd", 256),
                       ("c0", 5), ("c1", 5)):
        lay[name] = off
        off += cols
    return lay, off


def _mobius_C(resp, path_map):
    """Centered-basis Mobius coefficients C[t, jhi(4), jlo(8)]:
    out = sum_{jhi,jlo} C * q_jhi * m_jlo, with monomials of s_d = 2 p_d - 1
    (m: depths 0-2, jlo bit d set -> s_d factor; q: depths 3-4).
    Centering keeps |C| small so bf16 rounding of the monomials is not
    amplified by cancellation (validated: 2.8e-3 vs 1.7e-2 uncentered)."""
    path = np.asarray(path_map).reshape(NLEAF, D)
    assert np.all(path // 2 == np.arange(D)[None, :]), "non-oblivious path_map"
    bits = path & 1
    R = np.asarray(resp, np.float64).reshape(N, NLEAF)
    T = np.zeros((N, 2, 2, 2, 2, 2))
    for l in range(NLEAF):
        b = bits[l]
        T[:, b[0], b[1], b[2], b[3], b[4]] += R[:, l]
    for ax in range(1, 6):
        i0 = [slice(None)] * 6
        i1 = [slice(None)] * 6
        i0[ax], i1[ax] = 0, 1
        a0, a1 = T[tuple(i0)].copy(), T[tuple(i1)].copy()
        T[tuple(i0)] = 0.5 * (a0 + a1)
        T[tuple(i1)] = 0.5 * (a1 - a0)
    # axes t, b0..b4 -> [t, jhi=2*b4+b3, jlo=4*b2+2*b1+b0]
    return np.transpose(T, (0, 5, 4, 3, 2, 1)).reshape(N, 4, 8)


def _blob_offsets_mono():
    """Single packed input [128, W] (f32 cols; bf16 regions as col pairs).
    w = host-softmaxed choice weights (d-major), xt = host-transposed x."""
    off, lay = 0, {}
    for name, cols in (("w", 4 * ND // 2), ("xt", 4 * BC // 2),
                       ("cm", 512), ("onesd", 256), ("a", 5), ("c0", 5)):
        lay[name] = off
        off += cols
    return lay, off


def _build_nc_mono():
    nc = bass.Bass()
    lay, W = _blob_offsets_mono()
    blob = nc.dram_tensor("blob", [128, W], F32, kind="ExternalInput")
    out_d = nc.dram_tensor("out_t", [TC, BC], F32, kind="ExternalOutput")
    deps = []

    with tile.TileContext(nc) as tc:
        with (
            tc.tile_pool(name="const", bufs=1) as cpool,
            tc.tile_pool(name="big", bufs=1) as bpool,
            tc.tile_pool(name="work", bufs=2) as wpool,
            tc.tile_pool(name="out", bufs=1) as opool,
            tc.tile_pool(name="psum", bufs=3, space="PSUM") as pp,
            tc.tile_pool(name="psink", bufs=1, space="PSUM") as psink,
        ):
            sink = psink.tile([1, 1], F32, tag="sink")

            def pe_touch(ap):
                return nc.tensor.matmul(sink[:], ap, ap, start=True, stop=True,
                                        skip_group_check=True)

            def breg(name, cols):
                return blob[:, lay[name]:lay[name] + cols]

            # ---- inputs: spread across the two HWDGE queues (SP + Act),
            # (wt, xt) pairs first so mm1 ft-pass k starts as pair k lands
            wt = bpool.tile([128, 4 * ND], MMDT, tag="Wt")
            xts = bpool.tile([128, 4 * BC], MMDT, tag="xT")
            w0, x0 = lay["w"], lay["xt"]
            t_wt, t_xt = [], []
            for ft in range(4):
                nc.scalar.dma_start(
                    wt[:, ND * ft:ND * (ft + 1)],
                    blob[:, w0 + 320 * ft:w0 + 320 * (ft + 1)].bitcast(MMDT))
                nc.sync.dma_start(
                    xts[:, BC * ft:BC * (ft + 1)],
                    blob[:, x0 + 512 * ft:x0 + 512 * (ft + 1)].bitcast(MMDT))
                t_wt.append(pe_touch(wt[:, ND * ft:ND * ft + 1]))
                t_xt.append(pe_touch(xts[:, BC * ft:BC * ft + 1]))
            a_sb = cpool.tile([128, 5], F32, tag="a_sb")
            nc.scalar.dma_start(a_sb[:], breg("a", 5))
            c0t = cpool.tile([128, 5], F32, tag="c0")
            nc.scalar.dma_start(c0t[:], breg("c0", 5))
            onest = cpool.tile([128, 4 * 128], MMDT, tag="ones")
            nc.sync.dma_start(onest[:], breg("onesd", 256).bitcast(MMDT))
            t_on = pe_touch(onest[:, 0:1])
            cmt = cpool.tile([128, 8 * 128], MMDT, tag="cm")
            nc.sync.dma_start(cmt[:], breg("cm", 512).bitcast(MMDT))
            t_cm = pe_touch(cmt[:, 0:1])

            # ---- mm1 + gates: s_d = clip(a*FV + c0, -1, 1) ----
            # M [128 trees, 8*BC] bf16: lo-monomial block k (k bit d -> s_d);
            # H [128 trees, 4*BC] bf16: hi block j (1, s3, s4, s3*s4).
            # Depths 3,4 first so the H shuffle can overlap lo-depth mm1.
            M = bpool.tile([128, 8 * BC], MMDT, tag="M")
            H = bpool.tile([128, 4 * BC], MMDT, tag="H")
            nc.vector.memset(M[:, 0:BC], 1.0)
            nc.vector.memset(H[:, 0:BC], 1.0)
            kcol = {0: 1, 1: 2, 2: 4}   # depth -> lo-monomial block

            def mm1_gate(d):
                fv = pp.tile([128, BC], F32, tag="big", name=f"fv{d}")
                for ft in range(4):
                    st, sp = ft == 0, ft == 3
                    for h in range(2):
                        mm = nc.tensor.matmul(
                            fv[:, bass.ts(h, 512)],
                            wt[:, ND * ft + 128 * d:ND * ft + 128 * (d + 1)],
                            xts[:, BC * ft + 512 * h:BC * ft + 512 * (h + 1)],
                            start=st, stop=sp)
                        deps.extend([(mm, t_wt[ft]), (mm, t_xt[ft])])
                ut = wpool.tile([128, BC], F32, tag="u", name=f"u{d}")
                nc.vector.tensor_scalar(ut[:], fv[:], a_sb[:, d:d + 1],
                                        c0t[:, d:d + 1], ALU.mult, ALU.add)
                if d < 3:
                    dst = M[:, BC * kcol[d]:BC * (kcol[d] + 1)]
                else:
                    dst = H[:, BC * (d - 2):BC * (d - 1)]
                nc.vector.tensor_scalar(dst, ut[:], 1.0, -1.0,
                                        ALU.min, ALU.max)

            def mblk(k):
                return M[:, BC * k:BC * (k + 1)]

            # lo depths first: M completes while the hi-depth matmuls still
            # run on PE, so the Msh shuffle DMAs overlap mm1(d=3,4).
            mm1_gate(0)
            mm1_gate(1)
            nc.vector.tensor_mul(mblk(3), mblk(1), mblk(2))
            mm1_gate(2)
            nc.vector.tensor_mul(mblk(5), mblk(1), mblk(4))
            nc.vector.tensor_mul(mblk(6), mblk(2), mblk(4))
            nc.vector.tensor_mul(mblk(7), mblk(3), mblk(4))

            # ---- partition shuffle: tree-major -> block layout ----
            # Msh[sg][8*tl + k, b] = M[16*sg + tl, BC*k + b]  (flat row-major
            # copy of a [16, 8*BC] slice into [128, BC]); alternate the two
            # HWDGE queues so the 8 copies run in parallel pairs.  PE touches
            # are deferred past mm1(d=3,4) so PE does not stall on them.
            msh = [bpool.tile([128, BC], MMDT, tag=f"msh{sg}", name=f"msh{sg}")
                   for sg in range(8)]
            qsh = [bpool.tile([128, BC], MMDT, tag=f"qsh{al}", name=f"qsh{al}")
                   for al in range(4)]

            def shuf_m(sg):
                # gpsimd = software DGE: descriptors land in the parallel
                # hardware DMA rings instead of being moved by the (serial)
                # SP/Act sequencer DIRECT2D path
                nc.gpsimd.dma_start(msh[sg][:], M[16 * sg:16 * (sg + 1), :])

            def shuf_q(al):
                nc.gpsimd.dma_start(qsh[al][:], H[32 * al:32 * (al + 1), :])

            # DMA-queue order matches tail consumption: msh0-3 (z0, z1),
            # then qsh0/1 (P0, P1), then msh4-7 (z2, z3), then qsh2/3.
            for sg in (0, 1, 2, 3):
                shuf_m(sg)
            mm1_gate(3)
            mm1_gate(4)
            nc.vector.tensor_mul(H[:, 3 * BC:4 * BC], H[:, BC:2 * BC],
                                 H[:, 2 * BC:3 * BC])
            shuf_q(0)
            shuf_q(1)
            for sg in (4, 5, 6, 7):
                shuf_m(sg)
            shuf_q(2)
            shuf_q(3)
            t_msh = {}

            # ---- z = C . m (block-diag), P = Q * z, ones-reduce ----
            # PE emission order pipelines z(al+1) ahead of mm4(al) so PE
            # never stalls on the DVE P-multiply.
            outp = pp.tile([128, BC], F32, tag="big")
            zs = {}

            def emit_z(al):
                # touch only this pair's msh tiles: z(0) must not wait for
                # the later shuffle copies to land
                for sg in (2 * al, 2 * al + 1):
                    t_msh[sg] = pe_touch(msh[sg][:, 0:1])
                z = pp.tile([128, BC], F32, tag="big", name=f"z{al}")
                for j in range(2):
                    sg = 2 * al + j
                    for h in range(2):
                        mm = nc.tensor.matmul(
                            z[:, bass.ts(h, 512)],
                            cmt[:, bass.ts(sg, 128)],
                            msh[sg][:, bass.ts(h, 512)],
                            start=j == 0, stop=j == 1)
                        deps.extend([(mm, t_cm), (mm, t_msh[sg])])
                zs[al] = z

            emit_z(0)
            emit_z(1)
            for al in range(4):
                pt = wpool.tile([128, BC], MMDT, tag="P", name=f"pt{al}")
                nc.vector.tensor_mul(pt[:], qsh[al][:], zs[al][:])
                t_pt = pe_touch(pt[:, 0:1])
                if al + 2 <= 3:
                    emit_z(al + 2)
                for h in range(2):
                    mm = nc.tensor.matmul(
                        outp[:, bass.ts(h, 512)],
                        onest[:, bass.ts(al, 128)],
                        pt[:, bass.ts(h, 512)],
                        start=al == 0, stop=al == 3,
                        skip_group_check=True)
                    deps += [(mm, t_on), (mm, t_pt)]
            out_sb = opool.tile([128, BC], F32, tag="osb")
            nc.vector.tensor_copy(out_sb[:], outp[:])
            nc.sync.dma_start(out_d[:], out_sb[:])

            from concourse.tile import add_dep_helper
            for a, b in deps:
                add_dep_helper(a.ins, b.ins, sync=False,
                               reason="PE pre-sync absorbs extra waits")
    if FIXUP_WAITS:
        _split_excess_waits(nc)
    return nc


def _build_nc(npb, sb_passes, sa_passes):
    MMDT = F32R if USE_F32R else (BF16 if USE_BF16 else F32)
    lay, W = _blob_offsets(npb)
    nc = bass.Bass()
    blob = nc.dram_tensor("blob", [128, W], F32, kind="ExternalInput")

    def reg(name, cols, dt=F32):
        sl = blob[:, lay[name]:lay[name] + cols]
        return sl.bitcast(dt) if dt != F32 else sl

    out_d = nc.dram_tensor("out_t", [TC, BC], F32, kind="ExternalOutput")

    deps = []  # (dependent BassInstruction, dependency BassInstruction)

    with tile.TileContext(nc) as tc:
        with (
            tc.tile_pool(name="const", bufs=1) as cpool,
            tc.tile_pool(name="big", bufs=1) as bpool,
            tc.tile_pool(name="work", bufs=2) as wpool,
            tc.tile_pool(name="out", bufs=1) as opool,
            tc.tile_pool(name="psum", bufs=3, space="PSUM") as pp,
            tc.tile_pool(name="psink", bufs=1, space="PSUM") as psink,
        ):
            # PE matmuls (incl. transpose-mode) only tolerate ONE sync wait
            # after walrus lowering; "touch" matmuls absorb producer-engine
            # waits into PE's vector clock ahead of the real matmuls.
            sink = psink.tile([1, 1], F32, tag="sink")

            def pe_touch(ap):
                if ap.dtype == F32R:
                    ap = ap.bitcast(F32)
                return nc.tensor.matmul(sink[:], ap, ap, start=True, stop=True,
                                        skip_group_check=True)

            def mmop(ap):
                return ap.bitcast(F32) if MMDT == F32R else ap

            # ---- constants (one DMA each) ----
            idt = cpool.tile([128, 128], F32, tag="idt")
            masks.make_identity(nc, idt[:])
            t_idt = pe_touch(idt[:, 0:1])
            if MMDT == BF16:
                idm = cpool.tile([128, 128], MMDT, tag="idm")
                nc.vector.tensor_copy(idm[:], idt[:])
                t_idm = pe_touch(idm[:, 0:1])
            else:
                idm, t_idm = idt, t_idt
            assert MMDT == BF16, "blob layout assumes bf16 matmul dtype"
            onest = cpool.tile([128, 4 * 128], MMDT, tag="ones")
            nc.sync.dma_start(onest[:], reg("onesd", 256, MMDT))
            t_on = pe_touch(onest[:, 0:1])
            pbt = cpool.tile([128, npb * 128], MMDT, tag="pb")
            nc.sync.dma_start(pbt[:], reg("pb", npb * 64, MMDT))
            t_pb = pe_touch(pbt[:, 0:1])
            r2t = cpool.tile([128, 8 * 128], MMDT, tag="r2")
            nc.sync.dma_start(r2t[:], reg("r2l", 512, MMDT))
            t_r2 = pe_touch(r2t[:, 0:1])
            c0t = cpool.tile([128, 5], F32, tag="c0")
            nc.sync.dma_start(c0t[:], reg("c0", 5))
            c1t = cpool.tile([128, 5], F32, tag="c1")
            nc.sync.dma_start(c1t[:], reg("c1", 5))
            ones_col = cpool.tile([128, 1], MMDT, tag="onescol")
            ones_raw = cpool.tile([128, 1], F32, tag="onescolr")
            nc.vector.memset(ones_raw[:], 1.0)
            nc.vector.tensor_copy(ones_col[:], ones_raw[:])
            t_oc = pe_touch(ones_col[:])

            # ---- E = exp(A), column sums, a = c1/colsum ----
            et = bpool.tile([128, 4 * ND], MMDT, tag="E")
            fa0 = lay["fa"]
            for ft in range(4):
                araw = wpool.tile([128, ND], F32, tag="u")
                nc.sync.dma_start(araw[:],
                                  blob[:, fa0 + ND * ft:fa0 + ND * (ft + 1)])
                nc.scalar.activation(et[:, ND * ft:ND * (ft + 1)], araw[:], AF.Exp)
            cs_a = pp.tile([1, 512], F32, tag="big")
            cs_b = pp.tile([1, 128], F32, tag="big")
            for ft in range(4):
                st, sp = ft == 0, ft == 3
                m1 = nc.tensor.matmul(cs_a[:], mmop(ones_col[:]),
                                      mmop(et[:, ND * ft:ND * ft + 512]),
                                      start=st, stop=sp)
                m2 = nc.tensor.matmul(cs_b[:], mmop(ones_col[:]),
                                      mmop(et[:, ND * ft + 512:ND * (ft + 1)]),
                                      start=st, stop=sp)
                deps += [(m1, t_oc), (m2, t_oc)]
            invs = cpool.tile([1, ND], F32, tag="invs")
            nc.vector.reciprocal(invs[:, 0:512], cs_a[:])
            nc.vector.reciprocal(invs[:, 512:640], cs_b[:])
            a_sb = cpool.tile([128, 5], F32, tag="a_sb")
            for t in range(5):
                tp = pp.tile([128, 128], F32, tag="big")
                tr = nc.tensor.transpose(tp[:, 0:1], invs[0:1, bass.ts(t, 128)],
                                         idt[0:1, 0:1])
                deps.append((tr, t_idt))
                nc.vector.tensor_scalar(a_sb[:, t:t + 1], tp[:, 0:1],
                                        c1t[:, t:t + 1], None, ALU.mult)

            # ---- x^T via PE transpose ----
            xts = bpool.tile([128, 4 * BC], MMDT, tag="xT")
            xraw = bpool.tile([128, 8 * F], MMDT, tag="xraw")
            x0 = lay["x"]
            for bt in range(8 if PHASES >= 1 else 0):
                nc.sync.dma_start(
                    xraw[:, F * bt:F * (bt + 1)],
                    blob[:, x0 + 256 * bt:x0 + 256 * (bt + 1)].bitcast(MMDT))
            for ft in range(4 if PHASES >= 1 else 0):
                tp = pp.tile([128, BC], MMDT, tag="big")
                for bt in range(8):
                    tr = nc.tensor.transpose(
                        tp[:, bass.ts(bt, 128)],
                        xraw[:, F * bt + 128 * ft:F * bt + 128 * (ft + 1)],
                        idm[:])
                    deps.append((tr, t_idm))
                nc.vector.tensor_copy(xts[:, BC * ft:BC * (ft + 1)], tp[:])

            # ---- mm1 + gates + logs, per nd-tile ----
            lpt = bpool.tile([128, 5 * BC], MMDT, tag="Lp")
            lnt = bpool.tile([128, 5 * BC], MMDT, tag="Ln")
            t_u = []
            last_gate = None
            for t in range(5 if PHASES >= 1 else 0):
                fv = pp.tile([128, BC], F32, tag="big")
                for ft in range(4):
                    st, sp = ft == 0, ft == 3
                    for h in range(2):
                        nc.tensor.matmul(
                            fv[:, bass.ts(h, 512)],
                            et[:, ND * ft + 128 * t:ND * ft + 128 * (t + 1)],
                            xts[:, BC * ft + 512 * h:BC * ft + 512 * (h + 1)],
                            start=st, stop=sp)
                ut = wpool.tile([128, BC], F32, tag="u")
                nc.vector.tensor_scalar(ut[:], fv[:], a_sb[:, t:t + 1],
                                        c0t[:, t:t + 1], ALU.mult, ALU.add)
                t_u.append(pe_touch(ut[:, 0:1]))
                geng = nc.gpsimd if GATES_ON_GPSIMD else nc.vector
                upt = wpool.tile([128, BC], F32, tag="up")
                geng.tensor_scalar(upt[:], ut[:], 1.0, EPS, ALU.min, ALU.max)
                unt = wpool.tile([128, BC], F32, tag="un")
                geng.tensor_scalar(unt[:], ut[:], -1.0, 1.0,
                                   ALU.mult, ALU.add)
                geng.tensor_scalar(unt[:], unt[:], 1.0, EPS,
                                   ALU.min, ALU.max)
                if PHASES >= 2:
                    nc.scalar.activation(lpt[:, BC * t:BC * (t + 1)],
                                         upt[:], AF.Ln)
                    nc.scalar.activation(lnt[:, BC * t:BC * (t + 1)],
                                         unt[:], AF.Ln)
                last_gate = unt

            if PHASES < 3:
                out_sb = opool.tile([128, BC], F32, tag="osb")
                src_t = last_gate if PHASES < 2 else lpt
                if src_t is None:
                    nc.vector.memset(out_sb[:], 0.0)
                elif src_t is lpt:
                    nc.vector.tensor_copy(out_sb[:], src_t[:, 0:BC])
                else:
                    nc.vector.tensor_copy(out_sb[:], src_t[:])
                nc.sync.dma_start(out_d[:], out_sb[:])
                from concourse.tile import add_dep_helper as _adh
                for a, b in deps:
                    _adh(a.ins, b.ins, sync=False, reason="pre-sync")
                deps.clear()
            if PHASES < 3:
                pass  # skip back half entirely

            def lsrc(s, tau):
                src = lpt if s == 1 else lnt
                return src[:, BC * tau:BC * (tau + 1)]

            # ---- mm2 (path sums) + exp ----
            ebt = bpool.tile([128, 8 * BC], MMDT, tag="eB")
            for sg in range(8 if PHASES >= 3 else 0):
                sb = pp.tile([128, BC], F32, tag="big")
                passes = sb_passes[sg]
                for i, (pi, tau, s) in enumerate(passes):
                    st, sp = i == 0, i == len(passes) - 1
                    for h in range(2):
                        mm = nc.tensor.matmul(
                            sb[:, bass.ts(h, 512)],
                            pbt[:, bass.ts(pi, 128)],
                            lsrc(s, tau)[:, bass.ts(h, 512)],
                            start=st, stop=sp)
                        deps += [(mm, t_pb)] + [(mm, tu) for tu in t_u]
                nc.scalar.activation(ebt[:, BC * sg:BC * (sg + 1)], sb[:], AF.Exp)
            eat = bpool.tile([128, 4 * BC], F32, tag="eA")
            for al in range(4 if PHASES >= 3 else 0):
                sa = pp.tile([128, BC], F32, tag="big")
                passes = sa_passes[al]
                for i, (pi, tau, s) in enumerate(passes):
                    st, sp = i == 0, i == len(passes) - 1
                    for h in range(2):
                        mm = nc.tensor.matmul(
                            sa[:, bass.ts(h, 512)],
                            pbt[:, bass.ts(pi, 128)],
                            lsrc(s, tau)[:, bass.ts(h, 512)],
                            start=st, stop=sp)
                        deps += [(mm, t_pb)] + [(mm, tu) for tu in t_u]
                nc.scalar.activation(eat[:, BC * al:BC * (al + 1)], sa[:], AF.Exp)

            # ---- mm3 (z = R2 . e_lo), P = e_hi * z, mm4 (ones-reduce) ----
            if PHASES == 3:
                out_sb = opool.tile([128, BC], F32, tag="osb")
                nc.vector.tensor_copy(out_sb[:], ebt[:, 0:BC])
                nc.sync.dma_start(out_d[:], out_sb[:])
            outp = pp.tile([128, BC], F32, tag="big")
            t_prev = None
            for al in range(4 if PHASES >= 4 else 0):
                z = pp.tile([128, BC], F32, tag="big")
                for j in range(2):
                    sg = 2 * al + j
                    for h in range(2):
                        mm = nc.tensor.matmul(
                            z[:, bass.ts(h, 512)],
                            r2t[:, bass.ts(sg, 128)],
                            ebt[:, BC * sg + 512 * h:BC * sg + 512 * (h + 1)],
                            start=j == 0, stop=j == 1)
                        deps.append((mm, t_r2))
                        if t_prev is not None:
                            deps.append((mm, t_prev))
                pt = wpool.tile([128, BC], MMDT, tag="P")
                nc.vector.tensor_mul(pt[:], eat[:, BC * al:BC * (al + 1)], z[:])
                t_pt = pe_touch(pt[:, 0:1])
                for h in range(2):
                    mm = nc.tensor.matmul(
                        outp[:, bass.ts(h, 512)],
                        onest[:, bass.ts(al, 128)],
                        pt[:, bass.ts(h, 512)],
                        start=al == 0, stop=al == 3,
                        skip_group_check=True)
                    deps += [(mm, t_on), (mm, t_pt)]
                t_prev = t_pt
            if PHASES >= 4:
                out_sb = opool.tile([128, BC], F32, tag="osb")
                nc.vector.tensor_copy(out_sb[:], outp[:])
                nc.sync.dma_start(out_d[:], out_sb[:])

            from concourse.tile import add_dep_helper
            for a, b in deps:
                add_dep_helper(a.ins, b.ins, sync=False,
                               reason="PE pre-sync absorbs extra waits")
    if FIXUP_WAITS:
        _split_excess_waits(nc)
    return nc


def _split_excess_waits(nc):
    """Walrus codegen only fits ONE sync wait on PE Matmult and DMACopy
    instructions ("Too many sync wait commands").  Hoist the extras onto
    preceding same-engine InstEventSemaphore pseudos (one wait each), which
    the sequencer executes before the limited instruction."""
    exempt = {"InstEventSemaphore", "InstUnconditionalBranch",
              "InstISA", "InstHalt"}
    nfix = 0
    for fn in nc.m.functions:
        for bb in fn.blocks:
            il = bb.instructions
            out = []
            for inst in il:
                si = inst.sync_info
                lim = None if type(inst).__name__ in exempt else 1
                if si is not None and lim is not None and len(si.on_wait) > lim:
                    keep = list(si.on_wait[-lim:])
                    for w in si.on_wait[:-lim]:
                        nfix += 1
                        ev = mybir.InstEventSemaphore(
                            name=f"I-waitfix-{nfix}",
                            engine=inst.engine,
                            ins=[], outs=[],
                            sync_info=mybir.SyncInfo(on_wait=[w], on_update=[]),
                        )
                        ev.bass_nofuse = True
                        out.append(ev)
                    inst.sync_info = mybir.SyncInfo(
                        on_wait=keep, on_update=list(si.on_update))
                out.append(inst)
            il[:] = out
            assert len(bb.instructions) == len(out)
    return nfix


def _prep(path_map):
    key = (np.asarray(path_map).tobytes(), PHASES, GATES_ON_GPSIMD, USE_F32R,
           USE_BF16, DESIGN)
    if key not in _CACHE:
        if DESIGN == "mono":
            _CACHE[key] = (None, None, None, _build_nc_mono())
        else:
            P_lo, P_hi, leaf_hi, leaf_lo = _structure(path_map)
            pb, sb_passes, sa_passes = _mm2_passes(P_lo, P_hi)
            nc = _build_nc(pb.shape[0], sb_passes, sa_passes)
            _CACHE[key] = (pb, leaf_hi, leaf_lo, nc)
    return _CACHE[key]


def _onesd_pack():
    onesd = np.zeros((4, 128, 128), np.float32)
    for al in range(4):
        for tl in range(32):
            for hi in range(NHI):
                onesd[al, 4 * tl + hi, 32 * al + tl] = 1.0
    return np.ascontiguousarray(onesd.transpose(1, 0, 2).reshape(128, 512))


def _build_in_maps_mono(x, fa, thr, lt, resp, path_map, nc):
    mmnp = mybir.dt.np(BF16)
    lay, W = _blob_offsets_mono()
    fa64 = np.asarray(fa, np.float64)
    E = np.exp(fa64 - fa64.max(axis=0, keepdims=True))
    cw = (E / E.sum(axis=0, keepdims=True)).astype(np.float32)  # [F, N*D]
    invtemp = np.exp(-np.asarray(lt, np.float64))
    a_all = invtemp.astype(np.float32)                      # [N, D]
    c0_all = (-np.asarray(thr, np.float64) * invtemp).astype(np.float32)
    C = _mobius_C(resp, path_map)                           # [N, 4, 8]
    onesd = _onesd_pack()

    def asf32pairs(a):
        a = np.ascontiguousarray(a.astype(mmnp))
        return a.view(np.float32)

    in_maps = []
    for c in range(8):
        bi, ti = c // MT, c % MT
        t0 = ti * TC
        # d-major column order: tile d holds all TC trees at depth d
        cols = np.array([(t0 + t) * D + d for d in range(D)
                         for t in range(TC)])
        wc = np.ascontiguousarray(cw[:, cols])
        blob = np.zeros((128, W), np.float32)
        for ft in range(4):
            blob[:, lay["w"] + 320 * ft:lay["w"] + 320 * (ft + 1)] = \
                asf32pairs(wc[128 * ft:128 * (ft + 1)])
        xT = np.ascontiguousarray(x[bi * BC:(bi + 1) * BC].T)   # [F, BC]
        for ft in range(4):
            blob[:, lay["xt"] + 512 * ft:lay["xt"] + 512 * (ft + 1)] = \
                asf32pairs(xT[128 * ft:128 * (ft + 1)])
        cm8 = np.zeros((8, 128, 128), np.float32)
        for sg in range(8):
            off = 64 * (sg % 2)
            for tl in range(16):
                t = t0 + 16 * sg + tl
                for k in range(8):
                    for j in range(4):
                        cm8[sg, 8 * tl + k, off + 4 * tl + j] = C[t, j, k]
        cm2 = np.ascontiguousarray(cm8.transpose(1, 0, 2).reshape(128, 1024))
        blob[:, lay["cm"]:lay["cm"] + 512] = asf32pairs(cm2)
        blob[:, lay["onesd"]:lay["onesd"] + 256] = asf32pairs(onesd)
        blob[:, lay["a"]:lay["a"] + 5] = a_all[t0:t0 + TC]
        blob[:, lay["c0"]:lay["c0"] + 5] = c0_all[t0:t0 + TC]
        in_maps.append({"blob": blob})
    return in_maps, nc


def build_in_maps(x, feat_attention, feature_thresholds, log_temperatures,
                  response, path_map):
    x = np.ascontiguousarray(np.asarray(x, np.float32))
    fa = np.asarray(feat_attention, np.float32)
    thr = np.asarray(feature_thresholds, np.float32)
    lt = np.asarray(log_temperatures, np.float32)
    resp = np.asarray(response, np.float32).reshape(N, NLEAF)
    pb, leaf_hi, leaf_lo, nc = _prep(path_map)
    if DESIGN == "mono":
        return _build_in_maps_mono(x, fa, thr, lt, resp, path_map, nc)

    perm = _perm()
    invtemp = np.exp(-lt)                        # [N, D] host weight prep
    c1_all = (0.5 * invtemp).reshape(N * D)
    c0_all = (0.5 - 0.5 * thr * invtemp).reshape(N * D)
    # R2[n, hi, lo] = sum of response over leaves in that (hi, lo) group
    R2 = np.zeros((N, NHI, NLO), np.float32)
    np.add.at(R2, (slice(None), leaf_hi, leaf_lo), resp)

    onesd = np.zeros((4, 128, 128), np.float32)
    for al in range(4):
        for tl in range(32):
            for hi in range(NHI):
                onesd[al, 4 * tl + hi, 32 * al + tl] = 1.0
    onesd = np.ascontiguousarray(onesd.transpose(1, 0, 2).reshape(128, 512))
    pb2 = np.ascontiguousarray(
        pb.transpose(1, 0, 2).reshape(128, pb.shape[0] * 128))

    in_maps = []
    for c in range(8):
        bi, ti = c // MT, c % MT
        t0 = ti * TC
        cols = t0 * D + perm                      # permuted global nd columns
        c0 = np.ascontiguousarray(c0_all[cols].reshape(5, 128).T)
        c1 = np.ascontiguousarray(c1_all[cols].reshape(5, 128).T)
        r2l = np.zeros((8, 128, 128), np.float32)
        for sg in range(8):
            off = 64 * (sg % 2)
            for tl in range(16):
                t = t0 + 16 * sg + tl
                for hi in range(NHI):
                    for lo in range(NLO):
                        r2l[sg, 8 * tl + lo, off + 4 * tl + hi] = R2[t, hi, lo]
        r2l = np.ascontiguousarray(r2l.transpose(1, 0, 2).reshape(128, 1024))
        mmnp = mybir.dt.np(BF16)
        lay, W = _blob_offsets(pb.shape[0])

        def asf32pairs(a):
            """bf16 [128, 2k] -> f32-viewed [128, k] for blob packing."""
            a = np.ascontiguousarray(a.astype(mmnp))
            return a.view(np.float32)

        blob = np.zeros((128, W), np.float32)
        fac = np.ascontiguousarray(fa[:, t0 * D:(t0 + TC) * D][:, perm])
        for ft in range(4):
            blob[:, lay["fa"] + ND * ft:lay["fa"] + ND * (ft + 1)] = \
                fac[128 * ft:128 * (ft + 1)]
        xc = x[bi * BC:(bi + 1) * BC]
        for bt in range(8):
            blob[:, lay["x"] + 256 * bt:lay["x"] + 256 * (bt + 1)] = \
                asf32pairs(xc[128 * bt:128 * (bt + 1)])
        blob[:, lay["pb"]:lay["pb"] + pb.shape[0] * 64] = asf32pairs(pb2)
        blob[:, lay["r2l"]:lay["r2l"] + 512] = asf32pairs(r2l)
        blob[:, lay["onesd"]:lay["onesd"] + 256] = asf32pairs(onesd)
        blob[:, lay["c0"]:lay["c0"] + 5] = c0
        blob[:, lay["c1"]:lay["c1"] + 5] = c1
        in_maps.append({"blob": blob})
    return in_maps, nc


def kernel(x, feat_attention, feature_thresholds, log_temperatures,
           response, path_map):
    in_maps, nc = build_in_maps(x, feat_attention, feature_thresholds,
                                log_temperatures, response, path_map)
    res = run_bass_kernel_spmd(nc, in_maps, list(range(8)))
    global LAST
    LAST = res
    out = np.empty((B, N), np.float32)
    for c in range(8):
        bi, ti = c // MT, c % MT
        out[bi * BC:(bi + 1) * BC, ti * TC:(ti + 1) * TC] = res.results[c]["out_t"].T
    return out

